# revision 2
# baseline (speedup 1.0000x reference)
"""Batch-functional VGG16 forward on 8 Trainium2 NeuronCores.

Sharding: model_bs (4) x image-half (2) -> 8 cores. Each core runs the full
VGG forward for one model's weights on 16 of the 32 shared images.

Conv is 9-position implicit GEMM accumulated in PSUM (fp32r matmuls,
fp32 accumulate). Special packing for the small-Cin layers:
  - L0 (Cin=3):  dx-packed K=9 (3 shifted copies of x on partitions 0..8)
  - L1/L2 (Cin=64): position pairs packed into K=128 via a flat-shifted
    copy of the activations on partitions 64..127 (6 matmuls per chunk
    instead of 9).
Activations live in SBUF in zero-padded [P, (T), N, H+2, W+2] layout so a
conv tap (dy, dx) is just a base-offset shift of the rhs access pattern.
"""

import numpy as np
from contextlib import ExitStack

import concourse.bass as bass
import concourse.mybir as mybir
import concourse.tile as tile
from concourse.bass_utils import run_bass_kernel_spmd
from bass_rust import SyncInfo

f32 = mybir.dt.float32
f32r = mybir.dt.float32r
AF = mybir.ActivationFunctionType

N_CORES = 8
MODEL_BS, IMG_BS = 4, 32
N = 16  # images per core

# conv layers: (cin, cout, H_in, pool_after)
LAYERS = [
    (3, 64, 32, False),
    (64, 64, 32, True),
    (64, 128, 16, False),
    (128, 128, 16, True),
    (128, 256, 8, False),
    (256, 256, 8, False),
    (256, 256, 8, True),
    (256, 512, 4, False),
    (512, 512, 4, False),
    (512, 512, 4, True),
    (512, 512, 2, False),
    (512, 512, 2, False),
    (512, 512, 2, True),
]

# ---- wsmall blob column offsets (f32r, [128, WSMALL_COLS]) ----
W1_OFF = 0          # [9 rows, 3*64]   w1[c+3dx, dy*64+co]
W2P_OFF = 192       # [128, 3*64]      rows<64: w2[c,co,3dy]; rows>=64: 3dy+1
W2S_OFF = 384       # [64, 3*64]       w2[c,co,3dy+2]
W3P_OFF = 576       # [128, 3*128]
W3S_OFF = 960       # [64, 3*128]
W4_OFF = 1344       # [128, 9*128]
WFC3_OFF = 2496     # [128, 4*10]
WSMALL_COLS = 2536

# bias blob columns per layer (conv 0..12, then fc1, fc2)
_BCOLS = []
_c = 0
for (ci, co, h, p) in LAYERS:
    _BCOLS.append(_c)
    _c += max(1, co // 128)
BFC1_COL = _c
_c += 4
BFC2_COL = _c
_c += 4
BIAS_COLS = _c


def split_excess_waits(nc, max_regular=1, max_evsem=2):
    """This toolchain caps sync commands per instruction; spill extra waits
    onto preceding same-engine EventSemaphore ops."""
    uid = [0]
    for func in nc.m.functions:
        for block in func.blocks:
            new_insts = []
            for inst in block.instructions:
                si = inst.sync_info
                if si is not None and si.on_wait:
                    waits = list(si.on_wait)
                    cap = (
                        max_evsem
                        if isinstance(inst, mybir.InstEventSemaphore)
                        else max_regular
                    )
                    if len(waits) > cap:
                        keep, spill = waits[:cap], waits[cap:]
                        while spill:
                            batch, spill = spill[:max_evsem], spill[max_evsem:]
                            uid[0] += 1
                            new_insts.append(
                                mybir.InstEventSemaphore(
                                    name=f"waitspill_{uid[0]}",
                                    opcode="EventSemaphore",
                                    engine=inst.engine,
                                    sync_info=SyncInfo(on_wait=batch, on_update=[]),
                                    bass_nofuse=True,
                                )
                            )
                        inst.sync_info = SyncInfo(
                            on_wait=keep, on_update=list(si.on_update)
                        )
                new_insts.append(inst)
            block.instructions = new_insts
    return nc


def _flat(t):
    """Flat [P, free] view of a multi-dim SBUF tile."""
    shape = t.shape
    if len(shape) == 2:
        return t
    names = " ".join(f"d{i}" for i in range(1, len(shape)))
    return t.rearrange(f"p {names} -> p ({names})")


def emit(nc, tc, ctx, a):
    """Emit the whole per-core forward. `a` = dict of dram APs."""
    wbig = ctx.enter_context(tc.tile_pool(name="wbig", bufs=4))
    small = ctx.enter_context(tc.tile_pool(name="small", bufs=1))
    abuf = ctx.enter_context(tc.tile_pool(name="abuf", bufs=3))
    ptmp_pool = ctx.enter_context(tc.tile_pool(name="ptmp", bufs=1))
    ps = ctx.enter_context(tc.tile_pool(name="ps", bufs=6, space="PSUM"))
    psfc = ctx.enter_context(tc.tile_pool(name="psfc", bufs=1, space="PSUM"))

    # ---- persistent small tensors ----
    wsmall = small.tile([128, WSMALL_COLS], f32r, tag="wsmall")
    nc.sync.dma_start(wsmall[:], a["wsmall"])
    bias = small.tile([128, BIAS_COLS], f32, tag="bias")
    nc.sync.dma_start(bias[:], a["wbias"])
    bfc3 = small.tile([1, 10], f32r, tag="bfc3")
    nc.sync.dma_start(bfc3[:], a["bfc3"])
    ones = small.tile([1, 16], f32r, tag="ones")
    nc.gpsimd.memset(ones[:].bitcast(f32), 1.0)
    x3pair = small.tile([128, N, 18, 18], f32r, tag="x3pair")
    nc.gpsimd.memset(x3pair[:].bitcast(f32), 0.0)

    def relu_bias(dest_ap, psum_ap, layer_idx, tile_idx, parts):
        nc.scalar.activation(
            dest_ap,
            psum_ap,
            AF.Relu,
            bias=bias[0:parts, _BCOLS[layer_idx] + tile_idx : _BCOLS[layer_idx] + tile_idx + 1],
        )

    # ================= L0 + L1 + pool0 (grouped over 4 images) =================
    NG = 4
    prev_xpack = prev_y1 = None
    for g in range(IMG_BS // 2 // NG):
        imgs = range(g * NG, (g + 1) * NG)
        xpack = abuf.tile([9, NG, 34, 34], f32r, tag="abuf")
        nc.gpsimd.memset(xpack[:].bitcast(f32), 0.0)
        for i, n in enumerate(imgs):
            nc.scalar.dma_start(xpack[0:3, i, 1:33, 1:33], a["x"][:, n, :, :])
        L = NG * 34 * 34
        xf = _flat(xpack)
        nc.scalar.dma_start(xf[3:6, 0 : L - 1], xf[0:3, 1:L])
        nc.scalar.dma_start(xf[6:9, 0 : L - 2], xf[0:3, 2:L])

        y1 = abuf.tile([128, NG, 34, 34], f32r, tag="abuf")
        nc.gpsimd.memset(y1[:].bitcast(f32), 0.0)
        # L0: dx-packed K=9, 3 matmuls per (img, row-half) chunk
        for i in range(NG):
            for yh in range(2):
                y0 = yh * 16
                psum = ps.tile([64, 16, 32], f32, tag="ps")
                for dy in range(3):
                    nc.tensor.matmul(
                        psum[:],
                        wsmall[0:9, W1_OFF + dy * 64 : W1_OFF + (dy + 1) * 64],
                        xpack[0:9, i, y0 + dy : y0 + dy + 16, 0:32],
                        start=(dy == 0),
                        stop=(dy == 2),
                    )
                relu_bias(y1[0:64, i, 1 + y0 : 17 + y0, 1:33], psum[:], 0, 0, 64)
                # shifted copy for the L1 pair-packing (partitions 64..127)
                nc.scalar.dma_start(
                    y1[64:128, i, 1 + y0 : 17 + y0, 0:33],
                    y1[0:64, i, 1 + y0 : 17 + y0, 1:34],
                )

        y2 = abuf.tile([64, NG, 32, 32], f32r, tag="abuf")
        # L1: pair-packed, 6 matmuls per chunk
        for i in range(NG):
            for yh in range(2):
                y0 = yh * 16
                psum = ps.tile([64, 16, 32], f32, tag="ps")
                k = 0
                for dy in range(3):
                    nc.tensor.matmul(
                        psum[:],
                        wsmall[0:128, W2P_OFF + dy * 64 : W2P_OFF + (dy + 1) * 64],
                        y1[0:128, i, y0 + dy : y0 + dy + 16, 0:32],
                        start=(k == 0),
                        stop=False,
                    )
                    k += 1
                for dy in range(3):
                    nc.tensor.matmul(
                        psum[:],
                        wsmall[0:64, W2S_OFF + dy * 64 : W2S_OFF + (dy + 1) * 64],
                        y1[0:64, i, y0 + dy : y0 + dy + 16, 2:34],
                        start=False,
                        stop=(dy == 2),
                    )
                relu_bias(y2[0:64, i, y0 : y0 + 16, :], psum[:], 1, 0, 64)

        # pool0 into x3pair interior
        p0 = ptmp_pool.tile([64, NG, 32, 16], f32r, tag="ptmp")
        yv = y2.rearrange("p n h (w2 two) -> p n h w2 two", two=2)
        nc.vector.tensor_max(p0[:], yv[:, :, :, :, 0], yv[:, :, :, :, 1])
        pv = p0.rearrange("p n (h2 two) w2 -> p n h2 two w2", two=2)
        nc.vector.tensor_max(
            x3pair[0:64, g * NG : (g + 1) * NG, 1:17, 1:17],
            pv[:, :, :, 0, :],
            pv[:, :, :, 1, :],
        )

    # shifted copy for L2 pair-packing
    L3f = N * 18 * 18
    x3f = _flat(x3pair)
    nc.scalar.dma_start(x3f[64:128, 0 : L3f - 1], x3f[0:64, 1:L3f])

    # ================= L2 (pair-packed, 64->128) =================
    x4 = abuf.tile([128, N, 18, 18], f32r, tag="abuf")
    nc.gpsimd.memset(x4[:].bitcast(f32), 0.0)
    for c in range(8):
        n0 = c * 2
        psum = ps.tile([128, 2, 16, 16], f32, tag="ps")
        k = 0
        for dy in range(3):
            nc.tensor.matmul(
                psum[:],
                wsmall[0:128, W3P_OFF + dy * 128 : W3P_OFF + (dy + 1) * 128],
                x3pair[0:128, n0 : n0 + 2, dy : dy + 16, 0:16],
                start=(k == 0),
                stop=False,
            )
            k += 1
        for dy in range(3):
            nc.tensor.matmul(
                psum[:],
                wsmall[0:64, W3S_OFF + dy * 128 : W3S_OFF + (dy + 1) * 128],
                x3pair[0:64, n0 : n0 + 2, dy : dy + 16, 2:18],
                start=False,
                stop=(dy == 2),
            )
        relu_bias(x4[:, n0 : n0 + 2, 1:17, 1:17], psum[:], 2, 0, 128)

    # ================= L3 (128->128) =================
    y4 = abuf.tile([128, N, 16, 16], f32r, tag="abuf")
    for c in range(8):
        n0 = c * 2
        psum = ps.tile([128, 2, 16, 16], f32, tag="ps")
        for pos in range(9):
            dy, dx = pos // 3, pos % 3
            nc.tensor.matmul(
                psum[:],
                wsmall[0:128, W4_OFF + pos * 128 : W4_OFF + (pos + 1) * 128],
                x4[0:128, n0 : n0 + 2, dy : dy + 16, dx : dx + 16],
                start=(pos == 0),
                stop=(pos == 8),
            )
        relu_bias(y4[:, n0 : n0 + 2, :, :], psum[:], 3, 0, 128)

    # pool1 -> x5 padded [128, N, 10, 10]
    x5 = abuf.tile([128, N, 10, 10], f32r, tag="abuf")
    nc.gpsimd.memset(x5[:].bitcast(f32), 0.0)
    p1 = ptmp_pool.tile([128, N, 16, 8], f32r, tag="ptmp")
    yv = y4.rearrange("p n h (w2 two) -> p n h w2 two", two=2)
    nc.vector.tensor_max(p1[:], yv[:, :, :, :, 0], yv[:, :, :, :, 1])
    pv = p1.rearrange("p n (h2 two) w2 -> p n h2 two w2", two=2)
    nc.vector.tensor_max(x5[:, :, 1:9, 1:9], pv[:, :, :, 0, :], pv[:, :, :, 1, :])

    def load_wbig(name, t, cout):
        wt = wbig.tile([128, 9, cout], f32r, tag="wbig")
        nc.sync.dma_start(_flat(wt), a[name][:, t * 9 * cout : (t + 1) * 9 * cout])
        return wt

    def conv_mid(layer_idx, x_in, x_out, wname, chunks, Hs, pad_out, pool_spec=None):
        """Generic mid conv. x_in: [128, T_in, N?, Hp, Wp] padded (T_in may be 1
        -> no tile dim). chunks: list of (n0, cnt). Hs: output spatial."""
        cin, cout, H, _ = LAYERS[layer_idx]
        Tin, Tout = max(1, cin // 128), max(1, cout // 128)
        wts = [load_wbig(wname, t, cout) for t in range(Tin)]
        single_chunk = len(chunks) == 1
        if single_chunk:
            (n0, cnt) = chunks[0]
            psums = [
                ps.tile([128, cnt, H, H], f32, tag="ps", name=f"ps_l{layer_idx}_{to}")
                for to in range(Tout)
            ]
            nmm = Tin * 9
            for ti in range(Tin):
                for to in range(Tout):
                    for pos in range(9):
                        dy, dx = pos // 3, pos % 3
                        k = ti * 9 + pos
                        rhs = (
                            x_in[:, ti, n0 : n0 + cnt, dy : dy + H, dx : dx + H]
                            if Tin > 1 or len(x_in.shape) == 5
                            else x_in[:, n0 : n0 + cnt, dy : dy + H, dx : dx + H]
                        )
                        nc.tensor.matmul(
                            psums[to][:],
                            wts[ti][:, pos, to * 128 : (to + 1) * 128],
                            rhs,
                            start=(k == 0),
                            stop=(k == nmm - 1),
                        )
            for to in range(Tout):
                dest = (
                    x_out[:, to, n0 : n0 + cnt, 1 : 1 + Hs, 1 : 1 + Hs]
                    if pad_out
                    else x_out[:, to, n0 : n0 + cnt, :, :]
                )
                relu_bias(dest, psums[to][:], layer_idx, to, 128)
        else:
            for (n0, cnt) in chunks:
                for to in range(Tout):
                    psum = ps.tile([128, cnt, H, H], f32, tag="ps")
                    nmm = Tin * 9
                    for ti in range(Tin):
                        for pos in range(9):
                            dy, dx = pos // 3, pos % 3
                            k = ti * 9 + pos
                            rhs = (
                                x_in[:, ti, n0 : n0 + cnt, dy : dy + H, dx : dx + H]
                                if Tin > 1
                                else x_in[:, n0 : n0 + cnt, dy : dy + H, dx : dx + H]
                            )
                            nc.tensor.matmul(
                                psum[:],
                                wts[ti][:, pos, to * 128 : (to + 1) * 128],
                                rhs,
                                start=(k == 0),
                                stop=(k == nmm - 1),
                            )
                    dest = (
                        x_out[:, to, n0 : n0 + cnt, 1 : 1 + Hs, 1 : 1 + Hs]
                        if pad_out
                        else x_out[:, to, n0 : n0 + cnt, :, :]
                    )
                    relu_bias(dest, psum[:], layer_idx, to, 128)

    def pool_padded(y_in, x_out, T, H):
        """2x2 maxpool y_in [128, T, N, H, H] -> x_out [128, T, N, H/2+2, H/2+2] interior."""
        Hh = H // 2
        for t in range(T):
            pt = ptmp_pool.tile([128, N, H, Hh], f32r, tag="ptmp")
            yv = y_in.rearrange("p t n h (w2 two) -> p t n h w2 two", two=2)
            nc.vector.tensor_max(pt[:], yv[:, t, :, :, :, 0], yv[:, t, :, :, :, 1])
            pv = pt.rearrange("p n (h2 two) w2 -> p n h2 two w2", two=2)
            nc.vector.tensor_max(
                x_out[:, t, :, 1 : 1 + Hh, 1 : 1 + Hh],
                pv[:, :, :, 0, :],
                pv[:, :, :, 1, :],
            )

    # L4: 128->256 @8x8
    x6 = abuf.tile([128, 2, N, 10, 10], f32r, tag="abuf")
    nc.gpsimd.memset(x6[:].bitcast(f32), 0.0)
    conv_mid(4, x5, x6, "w_l4", [(0, 8), (8, 8)], 8, pad_out=True)
    # L5: 256->256
    x7 = abuf.tile([128, 2, N, 10, 10], f32r, tag="abuf")
    nc.gpsimd.memset(x7[:].bitcast(f32), 0.0)
    conv_mid(5, x6, x7, "w_l5", [(0, 8), (8, 8)], 8, pad_out=True)
    # L6: 256->256, then pool2
    y7 = abuf.tile([128, 2, N, 8, 8], f32r, tag="abuf")
    conv_mid(6, x7, y7, "w_l6", [(0, 8), (8, 8)], 8, pad_out=False)
    x8 = abuf.tile([128, 2, N, 6, 6], f32r, tag="abuf")
    nc.gpsimd.memset(x8[:].bitcast(f32), 0.0)
    pool_padded(y7, x8, 2, 8)

    # L7: 256->512 @4x4
    x9 = abuf.tile([128, 4, N, 6, 6], f32r, tag="abuf")
    nc.gpsimd.memset(x9[:].bitcast(f32), 0.0)
    conv_mid(7, x8, x9, "w_l7", [(0, 16)], 4, pad_out=True)
    # L8
    x10 = abuf.tile([128, 4, N, 6, 6], f32r, tag="abuf")
    nc.gpsimd.memset(x10[:].bitcast(f32), 0.0)
    conv_mid(8, x9, x10, "w_l8", [(0, 16)], 4, pad_out=True)
    # L9, then pool3
    y10 = abuf.tile([128, 4, N, 4, 4], f32r, tag="abuf")
    conv_mid(9, x10, y10, "w_l9", [(0, 16)], 4, pad_out=False)
    x11 = abuf.tile([128, 4, N, 4, 4], f32r, tag="abuf")
    nc.gpsimd.memset(x11[:].bitcast(f32), 0.0)
    pool_padded(y10, x11, 4, 4)

    # L10..L12 @2x2
    x12 = abuf.tile([128, 4, N, 4, 4], f32r, tag="abuf")
    nc.gpsimd.memset(x12[:].bitcast(f32), 0.0)
    conv_mid(10, x11, x12, "w_l10", [(0, 16)], 2, pad_out=True)
    x13 = abuf.tile([128, 4, N, 4, 4], f32r, tag="abuf")
    nc.gpsimd.memset(x13[:].bitcast(f32), 0.0)
    conv_mid(11, x12, x13, "w_l11", [(0, 16)], 2, pad_out=True)
    y13 = abuf.tile([128, 4, N, 2, 2], f32r, tag="abuf")
    conv_mid(12, x13, y13, "w_l12", [(0, 16)], 2, pad_out=False)

    # pool4 -> xfc [128, 4, 16]
    xfc = small.tile([128, 4, N], f32r, tag="xfc")
    for t in range(4):
        pt = ptmp_pool.tile([128, N, 2], f32r, tag="ptmp")
        nc.vector.tensor_max(pt[:], y13[:, t, :, :, 0], y13[:, t, :, :, 1])
        nc.vector.tensor_max(xfc[:, t, :], pt[:, :, 0], pt[:, :, 1])

    # FC1, FC2: out[dout, img]
    def fc_layer(x_in, wname, bias_col, out_tag):
        wt = wbig.tile([128, 4, 512], f32r, tag="wbig")
        nc.sync.dma_start(_flat(wt), a[wname][:, :])
        x_out = small.tile([128, 4, N], f32r, tag=out_tag)
        for to in range(4):
            psum = ps.tile([128, N], f32, tag="ps")
            for ti in range(4):
                nc.tensor.matmul(
                    psum[:],
                    wt[:, ti, to * 128 : (to + 1) * 128],
                    x_in[:, ti, :],
                    start=(ti == 0),
                    stop=(ti == 3),
                )
            nc.scalar.activation(
                x_out[:, to, :],
                psum[:],
                AF.Identity,
                bias=bias[:, bias_col + to : bias_col + to + 1],
            )
        return x_out

    xfc2 = fc_layer(xfc, "wfc1", BFC1_COL, "xfc2")
    xfc3 = fc_layer(xfc2, "wfc2", BFC2_COL, "xfc3")

    # FC3 (flipped): psum[img, dout] = sum_t xfc3[:,t,:].T @ wfc3_t + ones.T @ bfc3
    psum3 = psfc.tile([16, 10], f32, tag="psfc")
    for t in range(4):
        nc.tensor.matmul(
            psum3[:],
            xfc3[:, t, :],
            wsmall[0:128, WFC3_OFF + t * 10 : WFC3_OFF + (t + 1) * 10],
            start=(t == 0),
            stop=False,
        )
    nc.tensor.matmul(psum3[:], ones[0:1, 0:16], bfc3[0:1, 0:10], start=False, stop=True)
    outsb = small.tile([16, 10], f32, tag="outsb")
    nc.scalar.copy(outsb[:], psum3[:])
    nc.sync.dma_start(a["y"], outsb[:])


def build_nc(repeat=1):
    nc = bass.Bass("TRN2", target_bir_lowering=False, debug=False, num_devices=N_CORES)
    a = {}
    a["x"] = nc.dram_tensor("x", [3, N, 32, 32], f32r, kind="ExternalInput").ap()
    a["wsmall"] = nc.dram_tensor("wsmall", [128, WSMALL_COLS], f32r, kind="ExternalInput").ap()
    a["wbias"] = nc.dram_tensor("wbias", [128, BIAS_COLS], f32, kind="ExternalInput").ap()
    a["bfc3"] = nc.dram_tensor("bfc3", [1, 10], f32r, kind="ExternalInput").ap()
    for idx, cols in [(4, 1 * 9 * 256), (5, 2 * 9 * 256), (6, 2 * 9 * 256),
                      (7, 2 * 9 * 512), (8, 4 * 9 * 512), (9, 4 * 9 * 512),
                      (10, 4 * 9 * 512), (11, 4 * 9 * 512), (12, 4 * 9 * 512)]:
        a[f"w_l{idx}"] = nc.dram_tensor(f"w_l{idx}", [128, cols], f32r, kind="ExternalInput").ap()
    a["wfc1"] = nc.dram_tensor("wfc1", [128, 4 * 512], f32r, kind="ExternalInput").ap()
    a["wfc2"] = nc.dram_tensor("wfc2", [128, 4 * 512], f32r, kind="ExternalInput").ap()
    a["y"] = nc.dram_tensor("y", [N, 10], f32, kind="ExternalOutput").ap()

    with tile.TileContext(nc) as tc:
        with ExitStack() as ctx:
            if repeat > 1:
                with tc.For_i(0, repeat, 1):
                    emit(nc, tc, ctx, a)
            else:
                emit(nc, tc, ctx, a)
    split_excess_waits(nc)
    return nc


def prep_core_inputs(core, x, weights, biases):
    m = core // 2
    h = core % 2
    d = {}
    d["x"] = np.ascontiguousarray(
        np.asarray(x[h * N : (h + 1) * N]).transpose(1, 0, 2, 3)
    ).astype(np.float32)

    def W(j):
        return np.asarray(weights[j][m]).astype(np.float32)

    def B(j):
        return np.asarray(biases[j][m]).astype(np.float32)[:, 0]

    ws = np.zeros((128, WSMALL_COLS), np.float32)
    # L0: [9, 3, 64]: row c+3dx, col dy*64+co  <- w1[c, co, 3dy+dx]
    w1 = W(0).reshape(3, 64, 3, 3)  # [c, co, dy, dx]
    ws[0:9, W1_OFF : W1_OFF + 192] = (
        w1.transpose(3, 0, 2, 1).reshape(9, 192)
    )
    # L1 pair/single
    w2 = W(1).reshape(64, 64, 3, 3)
    pair = np.concatenate([w2[:, :, :, 0], w2[:, :, :, 1]], axis=0)  # [128, co, dy]
    ws[0:128, W2P_OFF : W2P_OFF + 192] = pair.transpose(0, 2, 1).reshape(128, 192)
    ws[0:64, W2S_OFF : W2S_OFF + 192] = (
        w2[:, :, :, 2].transpose(0, 2, 1).reshape(64, 192)
    )
    # L2 pair/single (cout=128)
    w3 = W(2).reshape(64, 128, 3, 3)
    pair = np.concatenate([w3[:, :, :, 0], w3[:, :, :, 1]], axis=0)
    ws[0:128, W3P_OFF : W3P_OFF + 384] = pair.transpose(0, 2, 1).reshape(128, 384)
    ws[0:64, W3S_OFF : W3S_OFF + 384] = (
        w3[:, :, :, 2].transpose(0, 2, 1).reshape(64, 384)
    )
    # L3: [128, 9*128]
    ws[0:128, W4_OFF : W4_OFF + 1152] = W(3).transpose(0, 2, 1).reshape(128, 1152)
    # FC3 weights [512, 10, 1] -> [128, 4, 10]
    wf3 = W(15)[:, :, 0].reshape(4, 128, 10).transpose(1, 0, 2)
    ws[0:128, WFC3_OFF : WFC3_OFF + 40] = wf3.reshape(128, 40)
    d["wsmall"] = ws

    bb = np.zeros((128, BIAS_COLS), np.float32)
    for j in range(13):
        co = LAYERS[j][1]
        t = max(1, co // 128)
        bb[: min(co, 128), _BCOLS[j] : _BCOLS[j] + t] = B(j).reshape(t, -1).T
    bb[:, BFC1_COL : BFC1_COL + 4] = B(13).reshape(4, 128).T
    bb[:, BFC2_COL : BFC2_COL + 4] = B(14).reshape(4, 128).T
    d["wbias"] = bb
    d["bfc3"] = B(15).reshape(1, 10)

    for j in range(4, 13):
        w = W(j)  # [cin, cout, 9]
        cin, cout = w.shape[0], w.shape[1]
        T = cin // 128
        blob = w.reshape(T, 128, cout, 9).transpose(1, 0, 3, 2)  # [128, T, 9, cout]
        d[f"w_l{j}"] = np.ascontiguousarray(blob.reshape(128, T * 9 * cout))

    for k, j in [("wfc1", 13), ("wfc2", 14)]:
        w = W(j)[:, :, 0]  # [512, 512]
        d[k] = np.ascontiguousarray(
            w.reshape(4, 128, 512).transpose(1, 0, 2).reshape(128, 4 * 512)
        )
    return d


def kernel(x, weights, biases):
    nc = build_nc()
    in_maps = [prep_core_inputs(c, x, weights, biases) for c in range(N_CORES)]
    res = run_bass_kernel_spmd(nc, in_maps, list(range(N_CORES)))
    out = np.zeros((MODEL_BS, IMG_BS, 10), np.float32)
    for c in range(N_CORES):
        m, h = c // 2, c % 2
        out[m, h * N : (h + 1) * N] = res.results[c]["y"]
    return out


# revision 4
# speedup vs baseline: 1.0085x; 1.0085x over previous
"""Batch-functional VGG16 forward on 8 Trainium2 NeuronCores.

Sharding: model_bs (4) x image-half (2) -> 8 cores. Each core runs the full
VGG forward for one model's weights on 16 of the 32 shared images.

Conv is 9-position implicit GEMM accumulated in PSUM (fp32r matmuls,
fp32 accumulate). Special packing for the small-Cin layers:
  - L0 (Cin=3):  dx-packed K=9 (3 shifted copies of x on partitions 0..8)
  - L1/L2 (Cin=64): position pairs packed into K=128 via a flat-shifted
    copy of the activations on partitions 64..127 (6 matmuls per chunk
    instead of 9).
Activations live in SBUF in zero-padded [P, (T), N, H+2, W+2] layout so a
conv tap (dy, dx) is just a base-offset shift of the rhs access pattern.
"""

import numpy as np
from contextlib import ExitStack

import concourse.bass as bass
import concourse.mybir as mybir
import concourse.tile as tile
from concourse.bass_utils import run_bass_kernel_spmd
from bass_rust import SyncInfo

f32 = mybir.dt.float32
f32r = mybir.dt.float32r
AF = mybir.ActivationFunctionType

N_CORES = 8
MODEL_BS, IMG_BS = 4, 32
N = 16  # images per core

# conv layers: (cin, cout, H_in, pool_after)
LAYERS = [
    (3, 64, 32, False),
    (64, 64, 32, True),
    (64, 128, 16, False),
    (128, 128, 16, True),
    (128, 256, 8, False),
    (256, 256, 8, False),
    (256, 256, 8, True),
    (256, 512, 4, False),
    (512, 512, 4, False),
    (512, 512, 4, True),
    (512, 512, 2, False),
    (512, 512, 2, False),
    (512, 512, 2, True),
]

# ---- wsmall blob column offsets (f32r, [128, WSMALL_COLS]) ----
W1_OFF = 0          # [9 rows, 3*64]   w1[c+3dx, dy*64+co]
W2P_OFF = 192       # [128, 3*64]      rows<64: w2[c,co,3dy]; rows>=64: 3dy+1
W2S_OFF = 384       # [64, 3*64]       w2[c,co,3dy+2]
W3P_OFF = 576       # [128, 3*128]
W3S_OFF = 960       # [64, 3*128]
W4_OFF = 1344       # [128, 9*128]
WFC3_OFF = 2496     # [128, 4*10]
WSMALL_COLS = 2536

# bias blob columns per layer (conv 0..12, then fc1, fc2)
_BCOLS = []
_c = 0
for (ci, co, h, p) in LAYERS:
    _BCOLS.append(_c)
    _c += max(1, co // 128)
BFC1_COL = _c
_c += 4
BFC2_COL = _c
_c += 4
BIAS_COLS = _c


def split_excess_waits(nc, max_regular=1, max_evsem=2):
    """This toolchain caps sync commands per instruction; spill extra waits
    onto preceding same-engine EventSemaphore ops."""
    uid = [0]
    for func in nc.m.functions:
        for block in func.blocks:
            new_insts = []
            for inst in block.instructions:
                si = inst.sync_info
                if si is not None and si.on_wait:
                    waits = list(si.on_wait)
                    cap = (
                        max_evsem
                        if isinstance(inst, mybir.InstEventSemaphore)
                        else max_regular
                    )
                    if len(waits) > cap:
                        keep, spill = waits[:cap], waits[cap:]
                        while spill:
                            batch, spill = spill[:max_evsem], spill[max_evsem:]
                            uid[0] += 1
                            new_insts.append(
                                mybir.InstEventSemaphore(
                                    name=f"waitspill_{uid[0]}",
                                    opcode="EventSemaphore",
                                    engine=inst.engine,
                                    sync_info=SyncInfo(on_wait=batch, on_update=[]),
                                    bass_nofuse=True,
                                )
                            )
                        inst.sync_info = SyncInfo(
                            on_wait=keep, on_update=list(si.on_update)
                        )
                new_insts.append(inst)
            block.instructions = new_insts
    return nc


def _flat(t):
    """Flat [P, free] view of a multi-dim SBUF tile."""
    shape = t.shape
    if len(shape) == 2:
        return t
    names = " ".join(f"d{i}" for i in range(1, len(shape)))
    return t.rearrange(f"p {names} -> p ({names})")


# timing-experiment modes: "full", "dma_only" (weight DMAs only),
# "static_weights" (matmuls read one resident dummy blob; no per-layer DMA)
MODE = "full"


def emit(nc, tc, ctx, a):
    """Emit the whole per-core forward. `a` = dict of dram APs."""
    wbig = ctx.enter_context(tc.tile_pool(name="wbig", bufs=4))
    small = ctx.enter_context(tc.tile_pool(name="small", bufs=1))
    abuf = ctx.enter_context(tc.tile_pool(name="abuf", bufs=3))
    ptmp_pool = ctx.enter_context(tc.tile_pool(name="ptmp", bufs=1))
    ps = ctx.enter_context(tc.tile_pool(name="ps", bufs=6, space="PSUM"))
    psfc = ctx.enter_context(tc.tile_pool(name="psfc", bufs=1, space="PSUM"))

    # ---- persistent small tensors ----
    wsmall = small.tile([128, WSMALL_COLS], f32r, tag="wsmall")
    nc.sync.dma_start(wsmall[:], a["wsmall"])
    bias = small.tile([128, BIAS_COLS], f32, tag="bias")
    nc.sync.dma_start(bias[:], a["wbias"])
    bfc3 = small.tile([1, 10], f32r, tag="bfc3")
    nc.sync.dma_start(bfc3[:], a["bfc3"])
    ones = small.tile([1, 16], f32r, tag="ones")
    nc.gpsimd.memset(ones[:].bitcast(f32), 1.0)
    x3pair = small.tile([128, N, 18, 18], f32r, tag="x3pair")
    nc.gpsimd.memset(x3pair[:].bitcast(f32), 0.0)

    def relu_bias(dest_ap, psum_ap, layer_idx, tile_idx, parts):
        nc.scalar.activation(
            dest_ap,
            psum_ap,
            AF.Relu,
            bias=bias[0:parts, _BCOLS[layer_idx] + tile_idx : _BCOLS[layer_idx] + tile_idx + 1],
        )

    # ================= L0 + L1 + pool0 (grouped over 4 images) =================
    NG = 4
    prev_xpack = prev_y1 = None
    for g in range(IMG_BS // 2 // NG):
        imgs = range(g * NG, (g + 1) * NG)
        xpack = abuf.tile([9, NG, 34, 34], f32r, tag="abuf")
        nc.gpsimd.memset(xpack[:].bitcast(f32), 0.0)
        for i, n in enumerate(imgs):
            nc.scalar.dma_start(xpack[0:3, i, 1:33, 1:33], a["x"][:, n, :, :])
        L = NG * 34 * 34
        xf = _flat(xpack)
        nc.scalar.dma_start(xf[3:6, 0 : L - 1], xf[0:3, 1:L])
        nc.scalar.dma_start(xf[6:9, 0 : L - 2], xf[0:3, 2:L])

        y1 = abuf.tile([128, NG, 34, 34], f32r, tag="abuf")
        nc.gpsimd.memset(y1[:].bitcast(f32), 0.0)
        # L0: dx-packed K=9, 3 matmuls per (img, row-half) chunk
        for i in range(NG):
            for yh in range(2):
                y0 = yh * 16
                psum = ps.tile([64, 16, 32], f32, tag="ps")
                for dy in range(3):
                    nc.tensor.matmul(
                        psum[:],
                        wsmall[0:9, W1_OFF + dy * 64 : W1_OFF + (dy + 1) * 64],
                        xpack[0:9, i, y0 + dy : y0 + dy + 16, 0:32],
                        start=(dy == 0),
                        stop=(dy == 2),
                    )
                relu_bias(y1[0:64, i, 1 + y0 : 17 + y0, 1:33], psum[:], 0, 0, 64)
                # shifted copy for the L1 pair-packing (partitions 64..127)
                nc.scalar.dma_start(
                    y1[64:128, i, 1 + y0 : 17 + y0, 0:33],
                    y1[0:64, i, 1 + y0 : 17 + y0, 1:34],
                )

        y2 = abuf.tile([64, NG, 32, 32], f32r, tag="abuf")
        # L1: pair-packed, 6 matmuls per chunk
        for i in range(NG):
            for yh in range(2):
                y0 = yh * 16
                psum = ps.tile([64, 16, 32], f32, tag="ps")
                k = 0
                for dy in range(3):
                    nc.tensor.matmul(
                        psum[:],
                        wsmall[0:128, W2P_OFF + dy * 64 : W2P_OFF + (dy + 1) * 64],
                        y1[0:128, i, y0 + dy : y0 + dy + 16, 0:32],
                        start=(k == 0),
                        stop=False,
                    )
                    k += 1
                for dy in range(3):
                    nc.tensor.matmul(
                        psum[:],
                        wsmall[0:64, W2S_OFF + dy * 64 : W2S_OFF + (dy + 1) * 64],
                        y1[0:64, i, y0 + dy : y0 + dy + 16, 2:34],
                        start=False,
                        stop=(dy == 2),
                    )
                relu_bias(y2[0:64, i, y0 : y0 + 16, :], psum[:], 1, 0, 64)

        # pool0 into x3pair interior
        p0 = ptmp_pool.tile([64, NG, 32, 16], f32r, tag="ptmp")
        yv = y2.rearrange("p n h (w2 two) -> p n h w2 two", two=2)
        nc.vector.tensor_max(p0[:], yv[:, :, :, :, 0], yv[:, :, :, :, 1])
        pv = p0.rearrange("p n (h2 two) w2 -> p n h2 two w2", two=2)
        nc.vector.tensor_max(
            x3pair[0:64, g * NG : (g + 1) * NG, 1:17, 1:17],
            pv[:, :, :, 0, :],
            pv[:, :, :, 1, :],
        )

    # shifted copy for L2 pair-packing
    L3f = N * 18 * 18
    x3f = _flat(x3pair)
    nc.scalar.dma_start(x3f[64:128, 0 : L3f - 1], x3f[0:64, 1:L3f])

    # ================= L2 (pair-packed, 64->128) =================
    x4 = abuf.tile([128, N, 18, 18], f32r, tag="abuf")
    nc.gpsimd.memset(x4[:].bitcast(f32), 0.0)
    for c in range(8):
        n0 = c * 2
        psum = ps.tile([128, 2, 16, 16], f32, tag="ps")
        k = 0
        for dy in range(3):
            nc.tensor.matmul(
                psum[:],
                wsmall[0:128, W3P_OFF + dy * 128 : W3P_OFF + (dy + 1) * 128],
                x3pair[0:128, n0 : n0 + 2, dy : dy + 16, 0:16],
                start=(k == 0),
                stop=False,
            )
            k += 1
        for dy in range(3):
            nc.tensor.matmul(
                psum[:],
                wsmall[0:64, W3S_OFF + dy * 128 : W3S_OFF + (dy + 1) * 128],
                x3pair[0:64, n0 : n0 + 2, dy : dy + 16, 2:18],
                start=False,
                stop=(dy == 2),
            )
        relu_bias(x4[:, n0 : n0 + 2, 1:17, 1:17], psum[:], 2, 0, 128)

    # ================= L3 (128->128) =================
    y4 = abuf.tile([128, N, 16, 16], f32r, tag="abuf")
    for c in range(8):
        n0 = c * 2
        psum = ps.tile([128, 2, 16, 16], f32, tag="ps")
        for pos in range(9):
            dy, dx = pos // 3, pos % 3
            nc.tensor.matmul(
                psum[:],
                wsmall[0:128, W4_OFF + pos * 128 : W4_OFF + (pos + 1) * 128],
                x4[0:128, n0 : n0 + 2, dy : dy + 16, dx : dx + 16],
                start=(pos == 0),
                stop=(pos == 8),
            )
        relu_bias(y4[:, n0 : n0 + 2, :, :], psum[:], 3, 0, 128)

    # pool1 -> x5 padded [128, N, 10, 10]
    x5 = abuf.tile([128, N, 10, 10], f32r, tag="abuf")
    nc.gpsimd.memset(x5[:].bitcast(f32), 0.0)
    p1 = ptmp_pool.tile([128, N, 16, 8], f32r, tag="ptmp")
    yv = y4.rearrange("p n h (w2 two) -> p n h w2 two", two=2)
    nc.vector.tensor_max(p1[:], yv[:, :, :, :, 0], yv[:, :, :, :, 1])
    pv = p1.rearrange("p n (h2 two) w2 -> p n h2 two w2", two=2)
    nc.vector.tensor_max(x5[:, :, 1:9, 1:9], pv[:, :, :, 0, :], pv[:, :, :, 1, :])

    static_w = None
    if MODE == "static_weights":
        static_w = small.tile([128, 9, 512], f32r, tag="static_w")
        nc.sync.dma_start(_flat(static_w), a["w_l8"][:, 0 : 9 * 512])

    def load_wbig(name, t, cout):
        if MODE == "static_weights":
            return static_w[:, :, 0:cout]
        wt = wbig.tile([128, 9, cout], f32r, tag="wbig")
        nc.sync.dma_start(_flat(wt), a[name][:, t * 9 * cout : (t + 1) * 9 * cout])
        return wt

    if MODE == "dma_only":
        # only the weight traffic: all big-layer blobs + fc, then a dummy out
        for idx, cout, T in [(4, 256, 1), (5, 256, 2), (6, 256, 2), (7, 512, 2),
                             (8, 512, 4), (9, 512, 4), (10, 512, 4), (11, 512, 4),
                             (12, 512, 4)]:
            for t in range(T):
                wt = wbig.tile([128, 9, cout], f32r, tag="wbig", name=f"dma_{idx}_{t}")
                nc.sync.dma_start(
                    _flat(wt), a[f"w_l{idx}"][:, t * 9 * cout : (t + 1) * 9 * cout]
                )
        for nm in ("wfc1", "wfc2"):
            wt = wbig.tile([128, 4, 512], f32r, tag="wbig", name=f"dma_{nm}")
            nc.sync.dma_start(_flat(wt), a[nm][:, :])
        outsb = small.tile([16, 10], f32, tag="outsb")
        nc.gpsimd.memset(outsb[:], 0.0)
        nc.sync.dma_start(a["y"], outsb[:])
        return

    def conv_mid(layer_idx, x_in, x_out, wname, chunks, Hs, pad_out, pool_spec=None):
        """Generic mid conv. x_in: [128, T_in, N?, Hp, Wp] padded (T_in may be 1
        -> no tile dim). chunks: list of (n0, cnt). Hs: output spatial."""
        cin, cout, H, _ = LAYERS[layer_idx]
        Tin, Tout = max(1, cin // 128), max(1, cout // 128)
        wts = [load_wbig(wname, t, cout) for t in range(Tin)]
        single_chunk = len(chunks) == 1
        if single_chunk:
            (n0, cnt) = chunks[0]
            psums = [
                ps.tile([128, cnt, H, H], f32, tag="ps", name=f"ps_l{layer_idx}_{to}")
                for to in range(Tout)
            ]
            nmm = Tin * 9
            for ti in range(Tin):
                for to in range(Tout):
                    for pos in range(9):
                        dy, dx = pos // 3, pos % 3
                        k = ti * 9 + pos
                        rhs = (
                            x_in[:, ti, n0 : n0 + cnt, dy : dy + H, dx : dx + H]
                            if Tin > 1 or len(x_in.shape) == 5
                            else x_in[:, n0 : n0 + cnt, dy : dy + H, dx : dx + H]
                        )
                        nc.tensor.matmul(
                            psums[to][:],
                            wts[ti][:, pos, to * 128 : (to + 1) * 128],
                            rhs,
                            start=(k == 0),
                            stop=(k == nmm - 1),
                        )
            for to in range(Tout):
                dest = (
                    x_out[:, to, n0 : n0 + cnt, 1 : 1 + Hs, 1 : 1 + Hs]
                    if pad_out
                    else x_out[:, to, n0 : n0 + cnt, :, :]
                )
                relu_bias(dest, psums[to][:], layer_idx, to, 128)
        else:
            for (n0, cnt) in chunks:
                for to in range(Tout):
                    psum = ps.tile([128, cnt, H, H], f32, tag="ps")
                    nmm = Tin * 9
                    for ti in range(Tin):
                        for pos in range(9):
                            dy, dx = pos // 3, pos % 3
                            k = ti * 9 + pos
                            rhs = (
                                x_in[:, ti, n0 : n0 + cnt, dy : dy + H, dx : dx + H]
                                if Tin > 1
                                else x_in[:, n0 : n0 + cnt, dy : dy + H, dx : dx + H]
                            )
                            nc.tensor.matmul(
                                psum[:],
                                wts[ti][:, pos, to * 128 : (to + 1) * 128],
                                rhs,
                                start=(k == 0),
                                stop=(k == nmm - 1),
                            )
                    dest = (
                        x_out[:, to, n0 : n0 + cnt, 1 : 1 + Hs, 1 : 1 + Hs]
                        if pad_out
                        else x_out[:, to, n0 : n0 + cnt, :, :]
                    )
                    relu_bias(dest, psum[:], layer_idx, to, 128)

    def pool_padded(y_in, x_out, T, H):
        """2x2 maxpool y_in [128, T, N, H, H] -> x_out [128, T, N, H/2+2, H/2+2] interior."""
        Hh = H // 2
        for t in range(T):
            pt = ptmp_pool.tile([128, N, H, Hh], f32r, tag="ptmp")
            yv = y_in.rearrange("p t n h (w2 two) -> p t n h w2 two", two=2)
            nc.vector.tensor_max(pt[:], yv[:, t, :, :, :, 0], yv[:, t, :, :, :, 1])
            pv = pt.rearrange("p n (h2 two) w2 -> p n h2 two w2", two=2)
            nc.vector.tensor_max(
                x_out[:, t, :, 1 : 1 + Hh, 1 : 1 + Hh],
                pv[:, :, :, 0, :],
                pv[:, :, :, 1, :],
            )

    # L4: 128->256 @8x8
    x6 = abuf.tile([128, 2, N, 10, 10], f32r, tag="abuf")
    nc.gpsimd.memset(x6[:].bitcast(f32), 0.0)
    conv_mid(4, x5, x6, "w_l4", [(0, 8), (8, 8)], 8, pad_out=True)
    # L5: 256->256
    x7 = abuf.tile([128, 2, N, 10, 10], f32r, tag="abuf")
    nc.gpsimd.memset(x7[:].bitcast(f32), 0.0)
    conv_mid(5, x6, x7, "w_l5", [(0, 8), (8, 8)], 8, pad_out=True)
    # L6: 256->256, then pool2
    y7 = abuf.tile([128, 2, N, 8, 8], f32r, tag="abuf")
    conv_mid(6, x7, y7, "w_l6", [(0, 8), (8, 8)], 8, pad_out=False)
    x8 = abuf.tile([128, 2, N, 6, 6], f32r, tag="abuf")
    nc.gpsimd.memset(x8[:].bitcast(f32), 0.0)
    pool_padded(y7, x8, 2, 8)

    # L7: 256->512 @4x4
    x9 = abuf.tile([128, 4, N, 6, 6], f32r, tag="abuf")
    nc.gpsimd.memset(x9[:].bitcast(f32), 0.0)
    conv_mid(7, x8, x9, "w_l7", [(0, 16)], 4, pad_out=True)
    # L8
    x10 = abuf.tile([128, 4, N, 6, 6], f32r, tag="abuf")
    nc.gpsimd.memset(x10[:].bitcast(f32), 0.0)
    conv_mid(8, x9, x10, "w_l8", [(0, 16)], 4, pad_out=True)
    # L9, then pool3
    y10 = abuf.tile([128, 4, N, 4, 4], f32r, tag="abuf")
    conv_mid(9, x10, y10, "w_l9", [(0, 16)], 4, pad_out=False)
    x11 = abuf.tile([128, 4, N, 4, 4], f32r, tag="abuf")
    nc.gpsimd.memset(x11[:].bitcast(f32), 0.0)
    pool_padded(y10, x11, 4, 4)

    # L10..L12 @2x2
    x12 = abuf.tile([128, 4, N, 4, 4], f32r, tag="abuf")
    nc.gpsimd.memset(x12[:].bitcast(f32), 0.0)
    conv_mid(10, x11, x12, "w_l10", [(0, 16)], 2, pad_out=True)
    x13 = abuf.tile([128, 4, N, 4, 4], f32r, tag="abuf")
    nc.gpsimd.memset(x13[:].bitcast(f32), 0.0)
    conv_mid(11, x12, x13, "w_l11", [(0, 16)], 2, pad_out=True)
    y13 = abuf.tile([128, 4, N, 2, 2], f32r, tag="abuf")
    conv_mid(12, x13, y13, "w_l12", [(0, 16)], 2, pad_out=False)

    # pool4 -> xfc [128, 4, 16]
    xfc = small.tile([128, 4, N], f32r, tag="xfc")
    for t in range(4):
        pt = ptmp_pool.tile([128, N, 2], f32r, tag="ptmp")
        nc.vector.tensor_max(pt[:], y13[:, t, :, :, 0], y13[:, t, :, :, 1])
        nc.vector.tensor_max(xfc[:, t, :], pt[:, :, 0], pt[:, :, 1])

    # FC1, FC2: out[dout, img]
    def fc_layer(x_in, wname, bias_col, out_tag):
        wt = wbig.tile([128, 4, 512], f32r, tag="wbig")
        nc.sync.dma_start(_flat(wt), a[wname][:, :])
        x_out = small.tile([128, 4, N], f32r, tag=out_tag)
        for to in range(4):
            psum = ps.tile([128, N], f32, tag="ps")
            for ti in range(4):
                nc.tensor.matmul(
                    psum[:],
                    wt[:, ti, to * 128 : (to + 1) * 128],
                    x_in[:, ti, :],
                    start=(ti == 0),
                    stop=(ti == 3),
                )
            nc.scalar.activation(
                x_out[:, to, :],
                psum[:],
                AF.Identity,
                bias=bias[:, bias_col + to : bias_col + to + 1],
            )
        return x_out

    xfc2 = fc_layer(xfc, "wfc1", BFC1_COL, "xfc2")
    xfc3 = fc_layer(xfc2, "wfc2", BFC2_COL, "xfc3")

    # FC3 (flipped): psum[img, dout] = sum_t xfc3[:,t,:].T @ wfc3_t + ones.T @ bfc3
    psum3 = psfc.tile([16, 10], f32, tag="psfc")
    for t in range(4):
        nc.tensor.matmul(
            psum3[:],
            xfc3[:, t, :],
            wsmall[0:128, WFC3_OFF + t * 10 : WFC3_OFF + (t + 1) * 10],
            start=(t == 0),
            stop=False,
        )
    nc.tensor.matmul(psum3[:], ones[0:1, 0:16], bfc3[0:1, 0:10], start=False, stop=True)
    outsb = small.tile([16, 10], f32, tag="outsb")
    nc.scalar.copy(outsb[:], psum3[:])
    nc.sync.dma_start(a["y"], outsb[:])


def build_nc(repeat=1):
    nc = bass.Bass("TRN2", target_bir_lowering=False, debug=False, num_devices=N_CORES)
    a = {}
    a["x"] = nc.dram_tensor("x", [3, N, 32, 32], f32r, kind="ExternalInput").ap()
    a["wsmall"] = nc.dram_tensor("wsmall", [128, WSMALL_COLS], f32r, kind="ExternalInput").ap()
    a["wbias"] = nc.dram_tensor("wbias", [128, BIAS_COLS], f32, kind="ExternalInput").ap()
    a["bfc3"] = nc.dram_tensor("bfc3", [1, 10], f32r, kind="ExternalInput").ap()
    for idx, cols in [(4, 1 * 9 * 256), (5, 2 * 9 * 256), (6, 2 * 9 * 256),
                      (7, 2 * 9 * 512), (8, 4 * 9 * 512), (9, 4 * 9 * 512),
                      (10, 4 * 9 * 512), (11, 4 * 9 * 512), (12, 4 * 9 * 512)]:
        a[f"w_l{idx}"] = nc.dram_tensor(f"w_l{idx}", [128, cols], f32r, kind="ExternalInput").ap()
    a["wfc1"] = nc.dram_tensor("wfc1", [128, 4 * 512], f32r, kind="ExternalInput").ap()
    a["wfc2"] = nc.dram_tensor("wfc2", [128, 4 * 512], f32r, kind="ExternalInput").ap()
    a["y"] = nc.dram_tensor("y", [N, 10], f32, kind="ExternalOutput").ap()

    with tile.TileContext(nc) as tc:
        with ExitStack() as ctx:
            if repeat > 1:
                with tc.For_i(0, repeat, 1):
                    emit(nc, tc, ctx, a)
            else:
                emit(nc, tc, ctx, a)
    split_excess_waits(nc)
    return nc


def prep_core_inputs(core, x, weights, biases):
    m = core // 2
    h = core % 2
    d = {}
    d["x"] = np.ascontiguousarray(
        np.asarray(x[h * N : (h + 1) * N]).transpose(1, 0, 2, 3)
    ).astype(np.float32)

    def W(j):
        return np.asarray(weights[j][m]).astype(np.float32)

    def B(j):
        return np.asarray(biases[j][m]).astype(np.float32)[:, 0]

    ws = np.zeros((128, WSMALL_COLS), np.float32)
    # L0: [9, 3, 64]: row c+3dx, col dy*64+co  <- w1[c, co, 3dy+dx]
    w1 = W(0).reshape(3, 64, 3, 3)  # [c, co, dy, dx]
    ws[0:9, W1_OFF : W1_OFF + 192] = (
        w1.transpose(3, 0, 2, 1).reshape(9, 192)
    )
    # L1 pair/single
    w2 = W(1).reshape(64, 64, 3, 3)
    pair = np.concatenate([w2[:, :, :, 0], w2[:, :, :, 1]], axis=0)  # [128, co, dy]
    ws[0:128, W2P_OFF : W2P_OFF + 192] = pair.transpose(0, 2, 1).reshape(128, 192)
    ws[0:64, W2S_OFF : W2S_OFF + 192] = (
        w2[:, :, :, 2].transpose(0, 2, 1).reshape(64, 192)
    )
    # L2 pair/single (cout=128)
    w3 = W(2).reshape(64, 128, 3, 3)
    pair = np.concatenate([w3[:, :, :, 0], w3[:, :, :, 1]], axis=0)
    ws[0:128, W3P_OFF : W3P_OFF + 384] = pair.transpose(0, 2, 1).reshape(128, 384)
    ws[0:64, W3S_OFF : W3S_OFF + 384] = (
        w3[:, :, :, 2].transpose(0, 2, 1).reshape(64, 384)
    )
    # L3: [128, 9*128]
    ws[0:128, W4_OFF : W4_OFF + 1152] = W(3).transpose(0, 2, 1).reshape(128, 1152)
    # FC3 weights [512, 10, 1] -> [128, 4, 10]
    wf3 = W(15)[:, :, 0].reshape(4, 128, 10).transpose(1, 0, 2)
    ws[0:128, WFC3_OFF : WFC3_OFF + 40] = wf3.reshape(128, 40)
    d["wsmall"] = ws

    bb = np.zeros((128, BIAS_COLS), np.float32)
    for j in range(13):
        co = LAYERS[j][1]
        t = max(1, co // 128)
        bb[: min(co, 128), _BCOLS[j] : _BCOLS[j] + t] = B(j).reshape(t, -1).T
    bb[:, BFC1_COL : BFC1_COL + 4] = B(13).reshape(4, 128).T
    bb[:, BFC2_COL : BFC2_COL + 4] = B(14).reshape(4, 128).T
    d["wbias"] = bb
    d["bfc3"] = B(15).reshape(1, 10)

    for j in range(4, 13):
        w = W(j)  # [cin, cout, 9]
        cin, cout = w.shape[0], w.shape[1]
        T = cin // 128
        blob = w.reshape(T, 128, cout, 9).transpose(1, 0, 3, 2)  # [128, T, 9, cout]
        d[f"w_l{j}"] = np.ascontiguousarray(blob.reshape(128, T * 9 * cout))

    for k, j in [("wfc1", 13), ("wfc2", 14)]:
        w = W(j)[:, :, 0]  # [512, 512]
        d[k] = np.ascontiguousarray(
            w.reshape(4, 128, 512).transpose(1, 0, 2).reshape(128, 4 * 512)
        )
    return d


def kernel(x, weights, biases):
    nc = build_nc()
    in_maps = [prep_core_inputs(c, x, weights, biases) for c in range(N_CORES)]
    res = run_bass_kernel_spmd(nc, in_maps, list(range(N_CORES)))
    out = np.zeros((MODEL_BS, IMG_BS, 10), np.float32)
    for c in range(N_CORES):
        m, h = c // 2, c % 2
        out[m, h * N : (h + 1) * N] = res.results[c]["y"]
    return out


# revision 6
# speedup vs baseline: 1.3071x; 1.2961x over previous
"""Batch-functional VGG16 forward on 8 Trainium2 NeuronCores.

Sharding: model_bs (4) x image-half (2) -> 8 cores. Each core runs the full
VGG forward for one model's weights on 16 of the 32 shared images.

Conv is 9-position implicit GEMM accumulated in PSUM (fp32r matmuls,
fp32 accumulate). Special packing for the small-Cin layers:
  - L0 (Cin=3):  dx-packed K=9 (3 shifted copies of x on partitions 0..8)
  - L1/L2 (Cin=64): position pairs packed into K=128 via a flat-shifted
    copy of the activations on partitions 64..127 (6 matmuls per chunk
    instead of 9).
Activations live in SBUF in zero-padded [P, (T), N, H+2, W+2] layout so a
conv tap (dy, dx) is just a base-offset shift of the rhs access pattern.
"""

import numpy as np
from contextlib import ExitStack

import concourse.bass as bass
import concourse.mybir as mybir
import concourse.tile as tile
from concourse.bass_utils import run_bass_kernel_spmd
from bass_rust import SyncInfo

f32 = mybir.dt.float32
f32r = mybir.dt.float16  # fp16 operands: pipelined LDWEIGHTS+FWL, half DMA
AF = mybir.ActivationFunctionType

N_CORES = 8
MODEL_BS, IMG_BS = 4, 32
N = 16  # images per core

# conv layers: (cin, cout, H_in, pool_after)
LAYERS = [
    (3, 64, 32, False),
    (64, 64, 32, True),
    (64, 128, 16, False),
    (128, 128, 16, True),
    (128, 256, 8, False),
    (256, 256, 8, False),
    (256, 256, 8, True),
    (256, 512, 4, False),
    (512, 512, 4, False),
    (512, 512, 4, True),
    (512, 512, 2, False),
    (512, 512, 2, False),
    (512, 512, 2, True),
]

# ---- wsmall blob column offsets (f32r, [128, WSMALL_COLS]) ----
W1_OFF = 0          # [9 rows, 3*64]   w1[c+3dx, dy*64+co]
W2P_OFF = 192       # [128, 3*64]      rows<64: w2[c,co,3dy]; rows>=64: 3dy+1
W2S_OFF = 384       # [64, 3*64]       w2[c,co,3dy+2]
W3P_OFF = 576       # [128, 3*128]
W3S_OFF = 960       # [64, 3*128]
W4_OFF = 1344       # [128, 9*128]
WFC3_OFF = 2496     # [128, 4*10]
WSMALL_COLS = 2536

# bias blob columns per layer (conv 0..12, then fc1, fc2)
_BCOLS = []
_c = 0
for (ci, co, h, p) in LAYERS:
    _BCOLS.append(_c)
    _c += max(1, co // 128)
BFC1_COL = _c
_c += 4
BFC2_COL = _c
_c += 4
BIAS_COLS = _c


def split_excess_waits(nc, max_regular=1, max_evsem=2):
    """This toolchain caps sync commands per instruction; spill extra waits
    onto preceding same-engine EventSemaphore ops."""
    uid = [0]
    for func in nc.m.functions:
        for block in func.blocks:
            new_insts = []
            for inst in block.instructions:
                si = inst.sync_info
                if si is not None and si.on_wait:
                    waits = list(si.on_wait)
                    cap = (
                        max_evsem
                        if isinstance(inst, mybir.InstEventSemaphore)
                        else max_regular
                    )
                    if len(waits) > cap:
                        keep, spill = waits[:cap], waits[cap:]
                        while spill:
                            batch, spill = spill[:max_evsem], spill[max_evsem:]
                            uid[0] += 1
                            new_insts.append(
                                mybir.InstEventSemaphore(
                                    name=f"waitspill_{uid[0]}",
                                    opcode="EventSemaphore",
                                    engine=inst.engine,
                                    sync_info=SyncInfo(on_wait=batch, on_update=[]),
                                    bass_nofuse=True,
                                )
                            )
                        inst.sync_info = SyncInfo(
                            on_wait=keep, on_update=list(si.on_update)
                        )
                new_insts.append(inst)
            block.instructions = new_insts
    return nc


def _flat(t):
    """Flat [P, free] view of a multi-dim SBUF tile."""
    shape = t.shape
    if len(shape) == 2:
        return t
    names = " ".join(f"d{i}" for i in range(1, len(shape)))
    return t.rearrange(f"p {names} -> p ({names})")


# timing-experiment modes: "full", "dma_only" (weight DMAs only),
# "static_weights" (matmuls read one resident dummy blob; no per-layer DMA)
MODE = "full"


def emit(nc, tc, ctx, a):
    """Emit the whole per-core forward. `a` = dict of dram APs."""
    wbig = ctx.enter_context(tc.tile_pool(name="wbig", bufs=4))
    small = ctx.enter_context(tc.tile_pool(name="small", bufs=1))
    abuf = ctx.enter_context(tc.tile_pool(name="abuf", bufs=3))
    ptmp_pool = ctx.enter_context(tc.tile_pool(name="ptmp", bufs=1))
    ps = ctx.enter_context(tc.tile_pool(name="ps", bufs=6, space="PSUM"))
    psfc = ctx.enter_context(tc.tile_pool(name="psfc", bufs=1, space="PSUM"))

    # ---- persistent small tensors ----
    wsmall = small.tile([128, WSMALL_COLS], f32r, tag="wsmall")
    nc.sync.dma_start(wsmall[:], a["wsmall"])
    bias = small.tile([128, BIAS_COLS], f32, tag="bias")
    nc.sync.dma_start(bias[:], a["wbias"])
    bfc3 = small.tile([1, 10], f32r, tag="bfc3")
    nc.sync.dma_start(bfc3[:], a["bfc3"])
    ones = small.tile([1, 16], f32r, tag="ones")
    nc.gpsimd.memset(ones[:], 1.0)
    x3pair = small.tile([128, N, 18, 18], f32r, tag="x3pair")
    nc.gpsimd.memset(x3pair[:], 0.0)

    def relu_bias(dest_ap, psum_ap, layer_idx, tile_idx, parts):
        nc.scalar.activation(
            dest_ap,
            psum_ap,
            AF.Relu,
            bias=bias[0:parts, _BCOLS[layer_idx] + tile_idx : _BCOLS[layer_idx] + tile_idx + 1],
        )

    # ================= L0 + L1 + pool0 (grouped over 4 images) =================
    NG = 4
    prev_xpack = prev_y1 = None
    for g in range(IMG_BS // 2 // NG):
        imgs = range(g * NG, (g + 1) * NG)
        xpack = abuf.tile([9, NG, 34, 34], f32r, tag="abuf")
        nc.gpsimd.memset(xpack[:], 0.0)
        for i, n in enumerate(imgs):
            nc.scalar.dma_start(xpack[0:3, i, 1:33, 1:33], a["x"][:, n, :, :])
        L = NG * 34 * 34
        xf = _flat(xpack)
        nc.scalar.dma_start(xf[3:6, 0 : L - 1], xf[0:3, 1:L])
        nc.scalar.dma_start(xf[6:9, 0 : L - 2], xf[0:3, 2:L])

        y1 = abuf.tile([128, NG, 34, 34], f32r, tag="abuf")
        nc.gpsimd.memset(y1[:], 0.0)
        # L0: dx-packed K=9, 3 matmuls per (img, row-half) chunk
        for i in range(NG):
            for yh in range(2):
                y0 = yh * 16
                psum = ps.tile([64, 16, 32], f32, tag="ps")
                for dy in range(3):
                    nc.tensor.matmul(
                        psum[:],
                        wsmall[0:9, W1_OFF + dy * 64 : W1_OFF + (dy + 1) * 64],
                        xpack[0:9, i, y0 + dy : y0 + dy + 16, 0:32],
                        start=(dy == 0),
                        stop=(dy == 2),
                    )
                relu_bias(y1[0:64, i, 1 + y0 : 17 + y0, 1:33], psum[:], 0, 0, 64)
                # shifted copy for the L1 pair-packing (partitions 64..127)
                nc.scalar.dma_start(
                    y1[64:128, i, 1 + y0 : 17 + y0, 0:33],
                    y1[0:64, i, 1 + y0 : 17 + y0, 1:34],
                )

        y2 = abuf.tile([64, NG, 32, 32], f32r, tag="abuf")
        # L1: pair-packed, 6 matmuls per chunk
        for i in range(NG):
            for yh in range(2):
                y0 = yh * 16
                psum = ps.tile([64, 16, 32], f32, tag="ps")
                k = 0
                for dy in range(3):
                    nc.tensor.matmul(
                        psum[:],
                        wsmall[0:128, W2P_OFF + dy * 64 : W2P_OFF + (dy + 1) * 64],
                        y1[0:128, i, y0 + dy : y0 + dy + 16, 0:32],
                        start=(k == 0),
                        stop=False,
                    )
                    k += 1
                for dy in range(3):
                    nc.tensor.matmul(
                        psum[:],
                        wsmall[0:64, W2S_OFF + dy * 64 : W2S_OFF + (dy + 1) * 64],
                        y1[0:64, i, y0 + dy : y0 + dy + 16, 2:34],
                        start=False,
                        stop=(dy == 2),
                    )
                relu_bias(y2[0:64, i, y0 : y0 + 16, :], psum[:], 1, 0, 64)

        # pool0 into x3pair interior
        p0 = ptmp_pool.tile([64, NG, 32, 16], f32r, tag="ptmp")
        yv = y2.rearrange("p n h (w2 two) -> p n h w2 two", two=2)
        nc.vector.tensor_max(p0[:], yv[:, :, :, :, 0], yv[:, :, :, :, 1])
        pv = p0.rearrange("p n (h2 two) w2 -> p n h2 two w2", two=2)
        nc.vector.tensor_max(
            x3pair[0:64, g * NG : (g + 1) * NG, 1:17, 1:17],
            pv[:, :, :, 0, :],
            pv[:, :, :, 1, :],
        )

    # shifted copy for L2 pair-packing
    L3f = N * 18 * 18
    x3f = _flat(x3pair)
    nc.scalar.dma_start(x3f[64:128, 0 : L3f - 1], x3f[0:64, 1:L3f])

    # ================= L2 (pair-packed, 64->128) =================
    x4 = abuf.tile([128, N, 18, 18], f32r, tag="abuf")
    nc.gpsimd.memset(x4[:], 0.0)
    for c in range(8):
        n0 = c * 2
        psum = ps.tile([128, 2, 16, 16], f32, tag="ps")
        k = 0
        for dy in range(3):
            nc.tensor.matmul(
                psum[:],
                wsmall[0:128, W3P_OFF + dy * 128 : W3P_OFF + (dy + 1) * 128],
                x3pair[0:128, n0 : n0 + 2, dy : dy + 16, 0:16],
                start=(k == 0),
                stop=False,
            )
            k += 1
        for dy in range(3):
            nc.tensor.matmul(
                psum[:],
                wsmall[0:64, W3S_OFF + dy * 128 : W3S_OFF + (dy + 1) * 128],
                x3pair[0:64, n0 : n0 + 2, dy : dy + 16, 2:18],
                start=False,
                stop=(dy == 2),
            )
        relu_bias(x4[:, n0 : n0 + 2, 1:17, 1:17], psum[:], 2, 0, 128)

    # ================= L3 (128->128) =================
    y4 = abuf.tile([128, N, 16, 16], f32r, tag="abuf")
    for c in range(8):
        n0 = c * 2
        psum = ps.tile([128, 2, 16, 16], f32, tag="ps")
        for pos in range(9):
            dy, dx = pos // 3, pos % 3
            nc.tensor.matmul(
                psum[:],
                wsmall[0:128, W4_OFF + pos * 128 : W4_OFF + (pos + 1) * 128],
                x4[0:128, n0 : n0 + 2, dy : dy + 16, dx : dx + 16],
                start=(pos == 0),
                stop=(pos == 8),
            )
        relu_bias(y4[:, n0 : n0 + 2, :, :], psum[:], 3, 0, 128)

    # pool1 -> x5 padded [128, N, 10, 10]
    x5 = abuf.tile([128, N, 10, 10], f32r, tag="abuf")
    nc.gpsimd.memset(x5[:], 0.0)
    p1 = ptmp_pool.tile([128, N, 16, 8], f32r, tag="ptmp")
    yv = y4.rearrange("p n h (w2 two) -> p n h w2 two", two=2)
    nc.vector.tensor_max(p1[:], yv[:, :, :, :, 0], yv[:, :, :, :, 1])
    pv = p1.rearrange("p n (h2 two) w2 -> p n h2 two w2", two=2)
    nc.vector.tensor_max(x5[:, :, 1:9, 1:9], pv[:, :, :, 0, :], pv[:, :, :, 1, :])

    static_w = None
    if MODE == "static_weights":
        static_w = small.tile([128, 9, 512], f32r, tag="static_w")
        nc.sync.dma_start(_flat(static_w), a["w_l8"][:, 0 : 9 * 512])

    def load_wbig(name, t, cout):
        if MODE == "static_weights":
            return static_w[:, :, 0:cout]
        wt = wbig.tile([128, 9, cout], f32r, tag="wbig")
        nc.sync.dma_start(_flat(wt), a[name][:, t * 9 * cout : (t + 1) * 9 * cout])
        return wt

    if MODE == "dma_only":
        # only the weight traffic: all big-layer blobs + fc, then a dummy out
        for idx, cout, T in [(4, 256, 1), (5, 256, 2), (6, 256, 2), (7, 512, 2),
                             (8, 512, 4), (9, 512, 4), (10, 512, 4), (11, 512, 4),
                             (12, 512, 4)]:
            for t in range(T):
                wt = wbig.tile([128, 9, cout], f32r, tag="wbig", name=f"dma_{idx}_{t}")
                nc.sync.dma_start(
                    _flat(wt), a[f"w_l{idx}"][:, t * 9 * cout : (t + 1) * 9 * cout]
                )
        for nm in ("wfc1", "wfc2"):
            wt = wbig.tile([128, 4, 512], f32r, tag="wbig", name=f"dma_{nm}")
            nc.sync.dma_start(_flat(wt), a[nm][:, :])
        outsb = small.tile([16, 10], f32, tag="outsb")
        nc.gpsimd.memset(outsb[:], 0.0)
        nc.sync.dma_start(a["y"], outsb[:])
        return

    def conv_mid(layer_idx, x_in, x_out, wname, chunks, Hs, pad_out, pool_spec=None):
        """Generic mid conv. x_in: [128, T_in, N?, Hp, Wp] padded (T_in may be 1
        -> no tile dim). chunks: list of (n0, cnt). Hs: output spatial."""
        cin, cout, H, _ = LAYERS[layer_idx]
        Tin, Tout = max(1, cin // 128), max(1, cout // 128)
        wts = [load_wbig(wname, t, cout) for t in range(Tin)]
        single_chunk = len(chunks) == 1
        if single_chunk:
            (n0, cnt) = chunks[0]
            psums = [
                ps.tile([128, cnt, H, H], f32, tag="ps", name=f"ps_l{layer_idx}_{to}")
                for to in range(Tout)
            ]
            nmm = Tin * 9
            for ti in range(Tin):
                for to in range(Tout):
                    for pos in range(9):
                        dy, dx = pos // 3, pos % 3
                        k = ti * 9 + pos
                        rhs = (
                            x_in[:, ti, n0 : n0 + cnt, dy : dy + H, dx : dx + H]
                            if Tin > 1 or len(x_in.shape) == 5
                            else x_in[:, n0 : n0 + cnt, dy : dy + H, dx : dx + H]
                        )
                        nc.tensor.matmul(
                            psums[to][:],
                            wts[ti][:, pos, to * 128 : (to + 1) * 128],
                            rhs,
                            start=(k == 0),
                            stop=(k == nmm - 1),
                        )
            for to in range(Tout):
                dest = (
                    x_out[:, to, n0 : n0 + cnt, 1 : 1 + Hs, 1 : 1 + Hs]
                    if pad_out
                    else x_out[:, to, n0 : n0 + cnt, :, :]
                )
                relu_bias(dest, psums[to][:], layer_idx, to, 128)
        else:
            for (n0, cnt) in chunks:
                for to in range(Tout):
                    psum = ps.tile([128, cnt, H, H], f32, tag="ps")
                    nmm = Tin * 9
                    for ti in range(Tin):
                        for pos in range(9):
                            dy, dx = pos // 3, pos % 3
                            k = ti * 9 + pos
                            rhs = (
                                x_in[:, ti, n0 : n0 + cnt, dy : dy + H, dx : dx + H]
                                if Tin > 1
                                else x_in[:, n0 : n0 + cnt, dy : dy + H, dx : dx + H]
                            )
                            nc.tensor.matmul(
                                psum[:],
                                wts[ti][:, pos, to * 128 : (to + 1) * 128],
                                rhs,
                                start=(k == 0),
                                stop=(k == nmm - 1),
                            )
                    dest = (
                        x_out[:, to, n0 : n0 + cnt, 1 : 1 + Hs, 1 : 1 + Hs]
                        if pad_out
                        else x_out[:, to, n0 : n0 + cnt, :, :]
                    )
                    relu_bias(dest, psum[:], layer_idx, to, 128)

    def pool_padded(y_in, x_out, T, H):
        """2x2 maxpool y_in [128, T, N, H, H] -> x_out [128, T, N, H/2+2, H/2+2] interior."""
        Hh = H // 2
        for t in range(T):
            pt = ptmp_pool.tile([128, N, H, Hh], f32r, tag="ptmp")
            yv = y_in.rearrange("p t n h (w2 two) -> p t n h w2 two", two=2)
            nc.vector.tensor_max(pt[:], yv[:, t, :, :, :, 0], yv[:, t, :, :, :, 1])
            pv = pt.rearrange("p n (h2 two) w2 -> p n h2 two w2", two=2)
            nc.vector.tensor_max(
                x_out[:, t, :, 1 : 1 + Hh, 1 : 1 + Hh],
                pv[:, :, :, 0, :],
                pv[:, :, :, 1, :],
            )

    # L4: 128->256 @8x8
    x6 = abuf.tile([128, 2, N, 10, 10], f32r, tag="abuf")
    nc.gpsimd.memset(x6[:], 0.0)
    conv_mid(4, x5, x6, "w_l4", [(0, 8), (8, 8)], 8, pad_out=True)
    # L5: 256->256
    x7 = abuf.tile([128, 2, N, 10, 10], f32r, tag="abuf")
    nc.gpsimd.memset(x7[:], 0.0)
    conv_mid(5, x6, x7, "w_l5", [(0, 8), (8, 8)], 8, pad_out=True)
    # L6: 256->256, then pool2
    y7 = abuf.tile([128, 2, N, 8, 8], f32r, tag="abuf")
    conv_mid(6, x7, y7, "w_l6", [(0, 8), (8, 8)], 8, pad_out=False)
    x8 = abuf.tile([128, 2, N, 6, 6], f32r, tag="abuf")
    nc.gpsimd.memset(x8[:], 0.0)
    pool_padded(y7, x8, 2, 8)

    # L7: 256->512 @4x4
    x9 = abuf.tile([128, 4, N, 6, 6], f32r, tag="abuf")
    nc.gpsimd.memset(x9[:], 0.0)
    conv_mid(7, x8, x9, "w_l7", [(0, 16)], 4, pad_out=True)
    # L8
    x10 = abuf.tile([128, 4, N, 6, 6], f32r, tag="abuf")
    nc.gpsimd.memset(x10[:], 0.0)
    conv_mid(8, x9, x10, "w_l8", [(0, 16)], 4, pad_out=True)
    # L9, then pool3
    y10 = abuf.tile([128, 4, N, 4, 4], f32r, tag="abuf")
    conv_mid(9, x10, y10, "w_l9", [(0, 16)], 4, pad_out=False)
    x11 = abuf.tile([128, 4, N, 4, 4], f32r, tag="abuf")
    nc.gpsimd.memset(x11[:], 0.0)
    pool_padded(y10, x11, 4, 4)

    # L10..L12 @2x2
    x12 = abuf.tile([128, 4, N, 4, 4], f32r, tag="abuf")
    nc.gpsimd.memset(x12[:], 0.0)
    conv_mid(10, x11, x12, "w_l10", [(0, 16)], 2, pad_out=True)
    x13 = abuf.tile([128, 4, N, 4, 4], f32r, tag="abuf")
    nc.gpsimd.memset(x13[:], 0.0)
    conv_mid(11, x12, x13, "w_l11", [(0, 16)], 2, pad_out=True)
    y13 = abuf.tile([128, 4, N, 2, 2], f32r, tag="abuf")
    conv_mid(12, x13, y13, "w_l12", [(0, 16)], 2, pad_out=False)

    # pool4 -> xfc [128, 4, 16]
    xfc = small.tile([128, 4, N], f32r, tag="xfc")
    for t in range(4):
        pt = ptmp_pool.tile([128, N, 2], f32r, tag="ptmp")
        nc.vector.tensor_max(pt[:], y13[:, t, :, :, 0], y13[:, t, :, :, 1])
        nc.vector.tensor_max(xfc[:, t, :], pt[:, :, 0], pt[:, :, 1])

    # FC1, FC2: out[dout, img]
    def fc_layer(x_in, wname, bias_col, out_tag):
        wt = wbig.tile([128, 4, 512], f32r, tag="wbig")
        nc.sync.dma_start(_flat(wt), a[wname][:, :])
        x_out = small.tile([128, 4, N], f32r, tag=out_tag)
        for to in range(4):
            psum = ps.tile([128, N], f32, tag="ps")
            for ti in range(4):
                nc.tensor.matmul(
                    psum[:],
                    wt[:, ti, to * 128 : (to + 1) * 128],
                    x_in[:, ti, :],
                    start=(ti == 0),
                    stop=(ti == 3),
                )
            nc.scalar.activation(
                x_out[:, to, :],
                psum[:],
                AF.Identity,
                bias=bias[:, bias_col + to : bias_col + to + 1],
            )
        return x_out

    xfc2 = fc_layer(xfc, "wfc1", BFC1_COL, "xfc2")
    xfc3 = fc_layer(xfc2, "wfc2", BFC2_COL, "xfc3")

    # FC3 (flipped): psum[img, dout] = sum_t xfc3[:,t,:].T @ wfc3_t + ones.T @ bfc3
    psum3 = psfc.tile([16, 10], f32, tag="psfc")
    for t in range(4):
        nc.tensor.matmul(
            psum3[:],
            xfc3[:, t, :],
            wsmall[0:128, WFC3_OFF + t * 10 : WFC3_OFF + (t + 1) * 10],
            start=(t == 0),
            stop=False,
        )
    nc.tensor.matmul(psum3[:], ones[0:1, 0:16], bfc3[0:1, 0:10], start=False, stop=True)
    outsb = small.tile([16, 10], f32, tag="outsb")
    nc.scalar.copy(outsb[:], psum3[:])
    nc.sync.dma_start(a["y"], outsb[:])


def build_nc(repeat=1):
    nc = bass.Bass("TRN2", target_bir_lowering=False, debug=False, num_devices=N_CORES)
    a = {}
    a["x"] = nc.dram_tensor("x", [3, N, 32, 32], f32r, kind="ExternalInput").ap()
    a["wsmall"] = nc.dram_tensor("wsmall", [128, WSMALL_COLS], f32r, kind="ExternalInput").ap()
    a["wbias"] = nc.dram_tensor("wbias", [128, BIAS_COLS], f32, kind="ExternalInput").ap()
    a["bfc3"] = nc.dram_tensor("bfc3", [1, 10], f32r, kind="ExternalInput").ap()
    for idx, cols in [(4, 1 * 9 * 256), (5, 2 * 9 * 256), (6, 2 * 9 * 256),
                      (7, 2 * 9 * 512), (8, 4 * 9 * 512), (9, 4 * 9 * 512),
                      (10, 4 * 9 * 512), (11, 4 * 9 * 512), (12, 4 * 9 * 512)]:
        a[f"w_l{idx}"] = nc.dram_tensor(f"w_l{idx}", [128, cols], f32r, kind="ExternalInput").ap()
    a["wfc1"] = nc.dram_tensor("wfc1", [128, 4 * 512], f32r, kind="ExternalInput").ap()
    a["wfc2"] = nc.dram_tensor("wfc2", [128, 4 * 512], f32r, kind="ExternalInput").ap()
    a["y"] = nc.dram_tensor("y", [N, 10], f32, kind="ExternalOutput").ap()

    with tile.TileContext(nc) as tc:
        with ExitStack() as ctx:
            if repeat > 1:
                with tc.For_i(0, repeat, 1):
                    emit(nc, tc, ctx, a)
            else:
                emit(nc, tc, ctx, a)
    split_excess_waits(nc)
    return nc


def prep_core_inputs(core, x, weights, biases):
    m = core // 2
    h = core % 2
    d = {}
    d["x"] = np.ascontiguousarray(
        np.asarray(x[h * N : (h + 1) * N]).transpose(1, 0, 2, 3)
    ).astype(np.float16)

    def W(j):
        return np.asarray(weights[j][m]).astype(np.float32)

    def B(j):
        return np.asarray(biases[j][m]).astype(np.float32)[:, 0]

    ws = np.zeros((128, WSMALL_COLS), np.float32)
    # L0: [9, 3, 64]: row c+3dx, col dy*64+co  <- w1[c, co, 3dy+dx]
    w1 = W(0).reshape(3, 64, 3, 3)  # [c, co, dy, dx]
    ws[0:9, W1_OFF : W1_OFF + 192] = (
        w1.transpose(3, 0, 2, 1).reshape(9, 192)
    )
    # L1 pair/single
    w2 = W(1).reshape(64, 64, 3, 3)
    pair = np.concatenate([w2[:, :, :, 0], w2[:, :, :, 1]], axis=0)  # [128, co, dy]
    ws[0:128, W2P_OFF : W2P_OFF + 192] = pair.transpose(0, 2, 1).reshape(128, 192)
    ws[0:64, W2S_OFF : W2S_OFF + 192] = (
        w2[:, :, :, 2].transpose(0, 2, 1).reshape(64, 192)
    )
    # L2 pair/single (cout=128)
    w3 = W(2).reshape(64, 128, 3, 3)
    pair = np.concatenate([w3[:, :, :, 0], w3[:, :, :, 1]], axis=0)
    ws[0:128, W3P_OFF : W3P_OFF + 384] = pair.transpose(0, 2, 1).reshape(128, 384)
    ws[0:64, W3S_OFF : W3S_OFF + 384] = (
        w3[:, :, :, 2].transpose(0, 2, 1).reshape(64, 384)
    )
    # L3: [128, 9*128]
    ws[0:128, W4_OFF : W4_OFF + 1152] = W(3).transpose(0, 2, 1).reshape(128, 1152)
    # FC3 weights [512, 10, 1] -> [128, 4, 10]
    wf3 = W(15)[:, :, 0].reshape(4, 128, 10).transpose(1, 0, 2)
    ws[0:128, WFC3_OFF : WFC3_OFF + 40] = wf3.reshape(128, 40)
    d["wsmall"] = ws.astype(np.float16)

    bb = np.zeros((128, BIAS_COLS), np.float32)
    for j in range(13):
        co = LAYERS[j][1]
        t = max(1, co // 128)
        bb[: min(co, 128), _BCOLS[j] : _BCOLS[j] + t] = B(j).reshape(t, -1).T
    bb[:, BFC1_COL : BFC1_COL + 4] = B(13).reshape(4, 128).T
    bb[:, BFC2_COL : BFC2_COL + 4] = B(14).reshape(4, 128).T
    d["wbias"] = bb
    d["bfc3"] = B(15).reshape(1, 10).astype(np.float16)

    for j in range(4, 13):
        w = W(j)  # [cin, cout, 9]
        cin, cout = w.shape[0], w.shape[1]
        T = cin // 128
        blob = w.reshape(T, 128, cout, 9).transpose(1, 0, 3, 2)  # [128, T, 9, cout]
        d[f"w_l{j}"] = np.ascontiguousarray(blob.reshape(128, T * 9 * cout)).astype(np.float16)

    for k, j in [("wfc1", 13), ("wfc2", 14)]:
        w = W(j)[:, :, 0]  # [512, 512]
        d[k] = np.ascontiguousarray(
            w.reshape(4, 128, 512).transpose(1, 0, 2).reshape(128, 4 * 512)
        ).astype(np.float16)
    return d


def kernel(x, weights, biases):
    nc = build_nc()
    in_maps = [prep_core_inputs(c, x, weights, biases) for c in range(N_CORES)]
    res = run_bass_kernel_spmd(nc, in_maps, list(range(N_CORES)))
    out = np.zeros((MODEL_BS, IMG_BS, 10), np.float32)
    for c in range(N_CORES):
        m, h = c // 2, c % 2
        out[m, h * N : (h + 1) * N] = res.results[c]["y"]
    return out


# revision 9
# speedup vs baseline: 1.4028x; 1.0732x over previous
"""Batch-functional VGG16 forward on 8 Trainium2 NeuronCores.

Sharding: model_bs (4) x image-half (2) -> 8 cores. Each core runs the full
VGG forward for one model's weights on 16 of the 32 shared images.

Conv is 9-position implicit GEMM accumulated in PSUM (fp32r matmuls,
fp32 accumulate). Special packing for the small-Cin layers:
  - L0 (Cin=3):  dx-packed K=9 (3 shifted copies of x on partitions 0..8)
  - L1/L2 (Cin=64): position pairs packed into K=128 via a flat-shifted
    copy of the activations on partitions 64..127 (6 matmuls per chunk
    instead of 9).
Activations live in SBUF in zero-padded [P, (T), N, H+2, W+2] layout so a
conv tap (dy, dx) is just a base-offset shift of the rhs access pattern.
"""

import numpy as np
from contextlib import ExitStack

import concourse.bass as bass
import concourse.mybir as mybir
import concourse.tile as tile
from concourse.bass_utils import run_bass_kernel_spmd
from bass_rust import SyncInfo

f32 = mybir.dt.float32
f32r = mybir.dt.float16  # fp16 operands: pipelined LDWEIGHTS+FWL, half DMA
AF = mybir.ActivationFunctionType

N_CORES = 8
MODEL_BS, IMG_BS = 4, 32
N = 16  # images per core

# conv layers: (cin, cout, H_in, pool_after)
LAYERS = [
    (3, 64, 32, False),
    (64, 64, 32, True),
    (64, 128, 16, False),
    (128, 128, 16, True),
    (128, 256, 8, False),
    (256, 256, 8, False),
    (256, 256, 8, True),
    (256, 512, 4, False),
    (512, 512, 4, False),
    (512, 512, 4, True),
    (512, 512, 2, False),
    (512, 512, 2, False),
    (512, 512, 2, True),
]

# ---- wsmall blob column offsets (f32r, [128, WSMALL_COLS]) ----
W1_OFF = 0          # [9 rows, 3*64]   w1[c+3dx, dy*64+co]
W2P_OFF = 192       # [128, 3*64]      rows<64: w2[c,co,3dy]; rows>=64: 3dy+1
W2S_OFF = 384       # [64, 3*64]       w2[c,co,3dy+2]
W3P_OFF = 576       # [128, 3*128]
W3S_OFF = 960       # [64, 3*128]
W4_OFF = 1344       # [128, 9*128]
WFC3_OFF = 2496     # [128, 4*10]
WSMALL_COLS = 2536

# bias blob columns per layer (conv 0..12, then fc1, fc2)
_BCOLS = []
_c = 0
for (ci, co, h, p) in LAYERS:
    _BCOLS.append(_c)
    _c += max(1, co // 128)
BFC1_COL = _c
_c += 4
BFC2_COL = _c
_c += 4
BIAS_COLS = _c


def split_excess_waits(nc, max_regular=1, max_evsem=2):
    """This toolchain caps sync commands per instruction; spill extra waits
    onto preceding same-engine EventSemaphore ops."""
    uid = [0]
    for func in nc.m.functions:
        for block in func.blocks:
            new_insts = []
            for inst in block.instructions:
                si = inst.sync_info
                if si is not None and si.on_wait:
                    waits = list(si.on_wait)
                    cap = (
                        max_evsem
                        if isinstance(inst, mybir.InstEventSemaphore)
                        else max_regular
                    )
                    if len(waits) > cap:
                        keep, spill = waits[:cap], waits[cap:]
                        while spill:
                            batch, spill = spill[:max_evsem], spill[max_evsem:]
                            uid[0] += 1
                            new_insts.append(
                                mybir.InstEventSemaphore(
                                    name=f"waitspill_{uid[0]}",
                                    opcode="EventSemaphore",
                                    engine=inst.engine,
                                    sync_info=SyncInfo(on_wait=batch, on_update=[]),
                                    bass_nofuse=True,
                                )
                            )
                        inst.sync_info = SyncInfo(
                            on_wait=keep, on_update=list(si.on_update)
                        )
                new_insts.append(inst)
            block.instructions = new_insts
    return nc


def _flat(t):
    """Flat [P, free] view of a multi-dim SBUF tile."""
    shape = t.shape
    if len(shape) == 2:
        return t
    names = " ".join(f"d{i}" for i in range(1, len(shape)))
    return t.rearrange(f"p {names} -> p ({names})")


# timing-experiment modes: "full", "dma_only" (weight DMAs only),
# "static_weights" (matmuls read one resident dummy blob; no per-layer DMA)
MODE = "full"
# PREFIX: stop after this checkpoint id (0..7); 99 = full kernel
PREFIX = 99


class _EarlyStop(Exception):
    pass


def _ckpt(nc, a, small, cid):
    if PREFIX <= cid:
        outsb = small.tile([16, 10], f32, tag="outsb", name="outsb_early")
        nc.gpsimd.memset(outsb[:], 0.0)
        nc.sync.dma_start(a["y"], outsb[:])
        raise _EarlyStop


def emit(nc, tc, ctx, a):
    """Emit the whole per-core forward. `a` = dict of dram APs."""
    wbig = ctx.enter_context(tc.tile_pool(name="wbig", bufs=4))
    small = ctx.enter_context(tc.tile_pool(name="small", bufs=1))
    abuf = ctx.enter_context(tc.tile_pool(name="abuf", bufs=4))
    ptmp_pool = ctx.enter_context(tc.tile_pool(name="ptmp", bufs=2))
    ps = ctx.enter_context(tc.tile_pool(name="ps", bufs=7, space="PSUM"))
    psfc = ctx.enter_context(tc.tile_pool(name="psfc", bufs=1, space="PSUM"))

    # ---- persistent small tensors ----
    wsmall = small.tile([128, WSMALL_COLS], f32r, tag="wsmall")
    nc.sync.dma_start(wsmall[:], a["wsmall"])
    bias = small.tile([128, BIAS_COLS], f32, tag="bias")
    nc.sync.dma_start(bias[:], a["wbias"])
    bfc3 = small.tile([1, 10], f32r, tag="bfc3")
    nc.sync.dma_start(bfc3[:], a["bfc3"])
    ones = small.tile([1, 16], f32r, tag="ones")
    nc.gpsimd.memset(ones[:], 1.0)
    x3pair = small.tile([128, N, 18, 18], f32r, tag="x3pair")
    nc.gpsimd.memset(x3pair[:], 0.0)

    def relu_bias(dest_ap, psum_ap, layer_idx, tile_idx, parts):
        nc.scalar.activation(
            dest_ap,
            psum_ap,
            AF.Relu,
            bias=bias[0:parts, _BCOLS[layer_idx] + tile_idx : _BCOLS[layer_idx] + tile_idx + 1],
        )

    # ================= L0 + L1 + pool0 (grouped over 4 images) =================
    NG = 4
    prev_xpack = prev_y1 = None
    for g in range(IMG_BS // 2 // NG):
        imgs = range(g * NG, (g + 1) * NG)
        xpack = abuf.tile([9, NG, 34, 34], f32r, tag="abuf")
        nc.gpsimd.memset(xpack[:], 0.0)
        for i, n in enumerate(imgs):
            # three column-shifted copies straight from DRAM (dx = 0, 1, 2)
            nc.scalar.dma_start(xpack[0:3, i, 1:33, 1:33], a["x"][:, n, :, :])
            nc.scalar.dma_start(xpack[3:6, i, 1:33, 0:32], a["x"][:, n, :, :])
            nc.scalar.dma_start(xpack[6:9, i, 1:33, 0:31], a["x"][:, n, :, 1:32])

        y1 = abuf.tile([128, NG, 34, 34], f32r, tag="abuf")
        nc.gpsimd.memset(y1[:], 0.0)
        # L0: dx-packed K=9, 3 matmuls per (img, row-half) chunk
        for i in range(NG):
            for yh in range(2):
                y0 = yh * 16
                psum = ps.tile([64, 16, 32], f32, tag="ps")
                for dy in range(3):
                    nc.tensor.matmul(
                        psum[:],
                        wsmall[0:9, W1_OFF + dy * 64 : W1_OFF + (dy + 1) * 64],
                        xpack[0:9, i, y0 + dy : y0 + dy + 16, 0:32],
                        start=(dy == 0),
                        stop=(dy == 2),
                    )
                relu_bias(y1[0:64, i, 1 + y0 : 17 + y0, 1:33], psum[:], 0, 0, 64)
                # pair-pack copy: same values, one column to the left (flat +1)
                relu_bias(y1[64:128, i, 1 + y0 : 17 + y0, 0:32], psum[:], 0, 0, 64)

        y2 = abuf.tile([64, NG, 32, 32], f32r, tag="abuf")
        # L1: pair-packed, 6 matmuls per chunk
        for i in range(NG):
            for yh in range(2):
                y0 = yh * 16
                psum = ps.tile([64, 16, 32], f32, tag="ps")
                k = 0
                for dy in range(3):
                    nc.tensor.matmul(
                        psum[:],
                        wsmall[0:128, W2P_OFF + dy * 64 : W2P_OFF + (dy + 1) * 64],
                        y1[0:128, i, y0 + dy : y0 + dy + 16, 0:32],
                        start=(k == 0),
                        stop=False,
                    )
                    k += 1
                for dy in range(3):
                    nc.tensor.matmul(
                        psum[:],
                        wsmall[0:64, W2S_OFF + dy * 64 : W2S_OFF + (dy + 1) * 64],
                        y1[0:64, i, y0 + dy : y0 + dy + 16, 2:34],
                        start=False,
                        stop=(dy == 2),
                    )
                relu_bias(y2[0:64, i, y0 : y0 + 16, :], psum[:], 1, 0, 64)

        # pool0 into x3pair interior
        p0 = ptmp_pool.tile([64, NG, 32, 16], f32r, tag="ptmp")
        yv = y2.rearrange("p n h (w2 two) -> p n h w2 two", two=2)
        nc.vector.tensor_max(p0[:], yv[:, :, :, :, 0], yv[:, :, :, :, 1])
        pv = p0.rearrange("p n (h2 two) w2 -> p n h2 two w2", two=2)
        nc.vector.tensor_max(
            x3pair[0:64, g * NG : (g + 1) * NG, 1:17, 1:17],
            pv[:, :, :, 0, :],
            pv[:, :, :, 1, :],
        )
        nc.vector.tensor_max(
            x3pair[64:128, g * NG : (g + 1) * NG, 1:17, 0:16],
            pv[:, :, :, 0, :],
            pv[:, :, :, 1, :],
        )

    # (x3pair partitions 64..127 are written directly by pool0 above)

    _ckpt(nc, a, small, 0)

    # ================= L2 (pair-packed, 64->128) =================
    x4 = abuf.tile([128, N, 18, 18], f32r, tag="abuf")
    nc.gpsimd.memset(x4[:], 0.0)
    for c in range(8):
        n0 = c * 2
        psum = ps.tile([128, 2, 16, 16], f32, tag="ps")
        k = 0
        for dy in range(3):
            nc.tensor.matmul(
                psum[:],
                wsmall[0:128, W3P_OFF + dy * 128 : W3P_OFF + (dy + 1) * 128],
                x3pair[0:128, n0 : n0 + 2, dy : dy + 16, 0:16],
                start=(k == 0),
                stop=False,
            )
            k += 1
        for dy in range(3):
            nc.tensor.matmul(
                psum[:],
                wsmall[0:64, W3S_OFF + dy * 128 : W3S_OFF + (dy + 1) * 128],
                x3pair[0:64, n0 : n0 + 2, dy : dy + 16, 2:18],
                start=False,
                stop=(dy == 2),
            )
        relu_bias(x4[:, n0 : n0 + 2, 1:17, 1:17], psum[:], 2, 0, 128)

    _ckpt(nc, a, small, 1)

    # ================= L3 (128->128) =================
    y4 = abuf.tile([128, N, 16, 16], f32r, tag="abuf")
    for c in range(8):
        n0 = c * 2
        psum = ps.tile([128, 2, 16, 16], f32, tag="ps")
        for pos in range(9):
            dy, dx = pos // 3, pos % 3
            nc.tensor.matmul(
                psum[:],
                wsmall[0:128, W4_OFF + pos * 128 : W4_OFF + (pos + 1) * 128],
                x4[0:128, n0 : n0 + 2, dy : dy + 16, dx : dx + 16],
                start=(pos == 0),
                stop=(pos == 8),
            )
        relu_bias(y4[:, n0 : n0 + 2, :, :], psum[:], 3, 0, 128)

    # pool1 -> x5 padded [128, N, 10, 10]
    x5 = abuf.tile([128, N, 10, 10], f32r, tag="abuf")
    nc.gpsimd.memset(x5[:], 0.0)
    p1 = ptmp_pool.tile([128, N, 16, 8], f32r, tag="ptmp")
    yv = y4.rearrange("p n h (w2 two) -> p n h w2 two", two=2)
    nc.vector.tensor_max(p1[:], yv[:, :, :, :, 0], yv[:, :, :, :, 1])
    pv = p1.rearrange("p n (h2 two) w2 -> p n h2 two w2", two=2)
    nc.vector.tensor_max(x5[:, :, 1:9, 1:9], pv[:, :, :, 0, :], pv[:, :, :, 1, :])

    static_w = None
    if MODE == "static_weights":
        static_w = small.tile([128, 9, 512], f32r, tag="static_w")
        nc.sync.dma_start(_flat(static_w), a["w_l8"][:, 0 : 9 * 512])

    _ckpt(nc, a, small, 2)

    def load_wbig(name, t, cout):
        if MODE == "static_weights":
            return static_w[:, :, 0:cout]
        wt = wbig.tile([128, 9, cout], f32r, tag="wbig")
        nc.sync.dma_start(_flat(wt), a[name][:, t * 9 * cout : (t + 1) * 9 * cout])
        return wt

    if MODE == "dma_only":
        # only the weight traffic: all big-layer blobs + fc, then a dummy out
        for idx, cout, T in [(4, 256, 1), (5, 256, 2), (6, 256, 2), (7, 512, 2),
                             (8, 512, 4), (9, 512, 4), (10, 512, 4), (11, 512, 4),
                             (12, 512, 4)]:
            for t in range(T):
                wt = wbig.tile([128, 9, cout], f32r, tag="wbig", name=f"dma_{idx}_{t}")
                nc.sync.dma_start(
                    _flat(wt), a[f"w_l{idx}"][:, t * 9 * cout : (t + 1) * 9 * cout]
                )
        for nm in ("wfc1", "wfc2"):
            wt = wbig.tile([128, 4, 512], f32r, tag="wbig", name=f"dma_{nm}")
            nc.sync.dma_start(_flat(wt), a[nm][:, :])
        outsb = small.tile([16, 10], f32, tag="outsb")
        nc.gpsimd.memset(outsb[:], 0.0)
        nc.sync.dma_start(a["y"], outsb[:])
        return

    def conv_mid(layer_idx, x_in, x_out, wname, chunks, Hs, pad_out, pool_spec=None):
        """Generic mid conv. x_in: [128, T_in, N?, Hp, Wp] padded (T_in may be 1
        -> no tile dim). chunks: list of (n0, cnt). Hs: output spatial."""
        cin, cout, H, _ = LAYERS[layer_idx]
        Tin, Tout = max(1, cin // 128), max(1, cout // 128)
        wts = [load_wbig(wname, t, cout) for t in range(Tin)]
        single_chunk = len(chunks) == 1
        if single_chunk:
            (n0, cnt) = chunks[0]
            psums = [
                ps.tile([128, cnt, H, H], f32, tag="ps", name=f"ps_l{layer_idx}_{to}")
                for to in range(Tout)
            ]
            nmm = Tin * 9
            for ti in range(Tin):
                for to in range(Tout):
                    for pos in range(9):
                        dy, dx = pos // 3, pos % 3
                        k = ti * 9 + pos
                        rhs = (
                            x_in[:, ti, n0 : n0 + cnt, dy : dy + H, dx : dx + H]
                            if Tin > 1 or len(x_in.shape) == 5
                            else x_in[:, n0 : n0 + cnt, dy : dy + H, dx : dx + H]
                        )
                        nc.tensor.matmul(
                            psums[to][:],
                            wts[ti][:, pos, to * 128 : (to + 1) * 128],
                            rhs,
                            start=(k == 0),
                            stop=(k == nmm - 1),
                        )
            for to in range(Tout):
                dest = (
                    x_out[:, to, n0 : n0 + cnt, 1 : 1 + Hs, 1 : 1 + Hs]
                    if pad_out
                    else x_out[:, to, n0 : n0 + cnt, :, :]
                )
                relu_bias(dest, psums[to][:], layer_idx, to, 128)
        else:
            for (n0, cnt) in chunks:
                for to in range(Tout):
                    psum = ps.tile([128, cnt, H, H], f32, tag="ps")
                    nmm = Tin * 9
                    for ti in range(Tin):
                        for pos in range(9):
                            dy, dx = pos // 3, pos % 3
                            k = ti * 9 + pos
                            rhs = (
                                x_in[:, ti, n0 : n0 + cnt, dy : dy + H, dx : dx + H]
                                if Tin > 1
                                else x_in[:, n0 : n0 + cnt, dy : dy + H, dx : dx + H]
                            )
                            nc.tensor.matmul(
                                psum[:],
                                wts[ti][:, pos, to * 128 : (to + 1) * 128],
                                rhs,
                                start=(k == 0),
                                stop=(k == nmm - 1),
                            )
                    dest = (
                        x_out[:, to, n0 : n0 + cnt, 1 : 1 + Hs, 1 : 1 + Hs]
                        if pad_out
                        else x_out[:, to, n0 : n0 + cnt, :, :]
                    )
                    relu_bias(dest, psum[:], layer_idx, to, 128)

    def pool_padded(y_in, x_out, T, H):
        """2x2 maxpool y_in [128, T, N, H, H] -> x_out [128, T, N, H/2+2, H/2+2] interior."""
        Hh = H // 2
        for t in range(T):
            pt = ptmp_pool.tile([128, N, H, Hh], f32r, tag="ptmp")
            yv = y_in.rearrange("p t n h (w2 two) -> p t n h w2 two", two=2)
            nc.vector.tensor_max(pt[:], yv[:, t, :, :, :, 0], yv[:, t, :, :, :, 1])
            pv = pt.rearrange("p n (h2 two) w2 -> p n h2 two w2", two=2)
            nc.vector.tensor_max(
                x_out[:, t, :, 1 : 1 + Hh, 1 : 1 + Hh],
                pv[:, :, :, 0, :],
                pv[:, :, :, 1, :],
            )

    # L4: 128->256 @8x8
    x6 = abuf.tile([128, 2, N, 10, 10], f32r, tag="abuf")
    nc.gpsimd.memset(x6[:], 0.0)
    conv_mid(4, x5, x6, "w_l4", [(0, 8), (8, 8)], 8, pad_out=True)
    # L5: 256->256
    x7 = abuf.tile([128, 2, N, 10, 10], f32r, tag="abuf")
    nc.gpsimd.memset(x7[:], 0.0)
    conv_mid(5, x6, x7, "w_l5", [(0, 8), (8, 8)], 8, pad_out=True)
    _ckpt(nc, a, small, 3)
    # L6: 256->256, then pool2
    y7 = abuf.tile([128, 2, N, 8, 8], f32r, tag="abuf")
    conv_mid(6, x7, y7, "w_l6", [(0, 8), (8, 8)], 8, pad_out=False)
    x8 = abuf.tile([128, 2, N, 6, 6], f32r, tag="abuf")
    nc.gpsimd.memset(x8[:], 0.0)
    pool_padded(y7, x8, 2, 8)

    _ckpt(nc, a, small, 4)
    # L7: 256->512 @4x4
    x9 = abuf.tile([128, 4, N, 6, 6], f32r, tag="abuf")
    nc.gpsimd.memset(x9[:], 0.0)
    conv_mid(7, x8, x9, "w_l7", [(0, 16)], 4, pad_out=True)
    # L8
    x10 = abuf.tile([128, 4, N, 6, 6], f32r, tag="abuf")
    nc.gpsimd.memset(x10[:], 0.0)
    conv_mid(8, x9, x10, "w_l8", [(0, 16)], 4, pad_out=True)
    # L9, then pool3
    y10 = abuf.tile([128, 4, N, 4, 4], f32r, tag="abuf")
    conv_mid(9, x10, y10, "w_l9", [(0, 16)], 4, pad_out=False)
    x11 = abuf.tile([128, 4, N, 4, 4], f32r, tag="abuf")
    nc.gpsimd.memset(x11[:], 0.0)
    pool_padded(y10, x11, 4, 4)

    _ckpt(nc, a, small, 5)
    # L10..L12 @2x2
    x12 = abuf.tile([128, 4, N, 4, 4], f32r, tag="abuf")
    nc.gpsimd.memset(x12[:], 0.0)
    conv_mid(10, x11, x12, "w_l10", [(0, 16)], 2, pad_out=True)
    x13 = abuf.tile([128, 4, N, 4, 4], f32r, tag="abuf")
    nc.gpsimd.memset(x13[:], 0.0)
    conv_mid(11, x12, x13, "w_l11", [(0, 16)], 2, pad_out=True)
    y13 = abuf.tile([128, 4, N, 2, 2], f32r, tag="abuf")
    conv_mid(12, x13, y13, "w_l12", [(0, 16)], 2, pad_out=False)

    # pool4 -> xfc [128, 4, 16]
    xfc = small.tile([128, 4, N], f32r, tag="xfc")
    for t in range(4):
        pt = ptmp_pool.tile([128, N, 2], f32r, tag="ptmp")
        nc.vector.tensor_max(pt[:], y13[:, t, :, :, 0], y13[:, t, :, :, 1])
        nc.vector.tensor_max(xfc[:, t, :], pt[:, :, 0], pt[:, :, 1])

    _ckpt(nc, a, small, 6)

    # FC1, FC2: out[dout, img]
    def fc_layer(x_in, wname, bias_col, out_tag):
        wt = wbig.tile([128, 4, 512], f32r, tag="wbig")
        nc.sync.dma_start(_flat(wt), a[wname][:, :])
        x_out = small.tile([128, 4, N], f32r, tag=out_tag)
        for to in range(4):
            psum = ps.tile([128, N], f32, tag="ps")
            for ti in range(4):
                nc.tensor.matmul(
                    psum[:],
                    wt[:, ti, to * 128 : (to + 1) * 128],
                    x_in[:, ti, :],
                    start=(ti == 0),
                    stop=(ti == 3),
                )
            nc.scalar.activation(
                x_out[:, to, :],
                psum[:],
                AF.Identity,
                bias=bias[:, bias_col + to : bias_col + to + 1],
            )
        return x_out

    xfc2 = fc_layer(xfc, "wfc1", BFC1_COL, "xfc2")
    xfc3 = fc_layer(xfc2, "wfc2", BFC2_COL, "xfc3")

    # FC3 (flipped): psum[img, dout] = sum_t xfc3[:,t,:].T @ wfc3_t + ones.T @ bfc3
    psum3 = psfc.tile([16, 10], f32, tag="psfc")
    for t in range(4):
        nc.tensor.matmul(
            psum3[:],
            xfc3[:, t, :],
            wsmall[0:128, WFC3_OFF + t * 10 : WFC3_OFF + (t + 1) * 10],
            start=(t == 0),
            stop=False,
        )
    nc.tensor.matmul(psum3[:], ones[0:1, 0:16], bfc3[0:1, 0:10], start=False, stop=True)
    outsb = small.tile([16, 10], f32, tag="outsb")
    nc.scalar.copy(outsb[:], psum3[:])
    nc.sync.dma_start(a["y"], outsb[:])


def build_nc(repeat=1):
    nc = bass.Bass("TRN2", target_bir_lowering=False, debug=False, num_devices=N_CORES)
    a = {}
    a["x"] = nc.dram_tensor("x", [3, N, 32, 32], f32r, kind="ExternalInput").ap()
    a["wsmall"] = nc.dram_tensor("wsmall", [128, WSMALL_COLS], f32r, kind="ExternalInput").ap()
    a["wbias"] = nc.dram_tensor("wbias", [128, BIAS_COLS], f32, kind="ExternalInput").ap()
    a["bfc3"] = nc.dram_tensor("bfc3", [1, 10], f32r, kind="ExternalInput").ap()
    for idx, cols in [(4, 1 * 9 * 256), (5, 2 * 9 * 256), (6, 2 * 9 * 256),
                      (7, 2 * 9 * 512), (8, 4 * 9 * 512), (9, 4 * 9 * 512),
                      (10, 4 * 9 * 512), (11, 4 * 9 * 512), (12, 4 * 9 * 512)]:
        a[f"w_l{idx}"] = nc.dram_tensor(f"w_l{idx}", [128, cols], f32r, kind="ExternalInput").ap()
    a["wfc1"] = nc.dram_tensor("wfc1", [128, 4 * 512], f32r, kind="ExternalInput").ap()
    a["wfc2"] = nc.dram_tensor("wfc2", [128, 4 * 512], f32r, kind="ExternalInput").ap()
    a["y"] = nc.dram_tensor("y", [N, 10], f32, kind="ExternalOutput").ap()

    with tile.TileContext(nc) as tc:
        with ExitStack() as ctx:
            def emit_safe():
                try:
                    emit(nc, tc, ctx, a)
                except _EarlyStop:
                    pass

            if repeat > 1:
                with tc.For_i(0, repeat, 1):
                    emit_safe()
            else:
                emit_safe()
    split_excess_waits(nc)
    return nc


def prep_core_inputs(core, x, weights, biases):
    m = core // 2
    h = core % 2
    d = {}
    d["x"] = np.ascontiguousarray(
        np.asarray(x[h * N : (h + 1) * N]).transpose(1, 0, 2, 3)
    ).astype(np.float16)

    def W(j):
        return np.asarray(weights[j][m]).astype(np.float32)

    def B(j):
        return np.asarray(biases[j][m]).astype(np.float32)[:, 0]

    ws = np.zeros((128, WSMALL_COLS), np.float32)
    # L0: [9, 3, 64]: row c+3dx, col dy*64+co  <- w1[c, co, 3dy+dx]
    w1 = W(0).reshape(3, 64, 3, 3)  # [c, co, dy, dx]
    ws[0:9, W1_OFF : W1_OFF + 192] = (
        w1.transpose(3, 0, 2, 1).reshape(9, 192)
    )
    # L1 pair/single
    w2 = W(1).reshape(64, 64, 3, 3)
    pair = np.concatenate([w2[:, :, :, 0], w2[:, :, :, 1]], axis=0)  # [128, co, dy]
    ws[0:128, W2P_OFF : W2P_OFF + 192] = pair.transpose(0, 2, 1).reshape(128, 192)
    ws[0:64, W2S_OFF : W2S_OFF + 192] = (
        w2[:, :, :, 2].transpose(0, 2, 1).reshape(64, 192)
    )
    # L2 pair/single (cout=128)
    w3 = W(2).reshape(64, 128, 3, 3)
    pair = np.concatenate([w3[:, :, :, 0], w3[:, :, :, 1]], axis=0)
    ws[0:128, W3P_OFF : W3P_OFF + 384] = pair.transpose(0, 2, 1).reshape(128, 384)
    ws[0:64, W3S_OFF : W3S_OFF + 384] = (
        w3[:, :, :, 2].transpose(0, 2, 1).reshape(64, 384)
    )
    # L3: [128, 9*128]
    ws[0:128, W4_OFF : W4_OFF + 1152] = W(3).transpose(0, 2, 1).reshape(128, 1152)
    # FC3 weights [512, 10, 1] -> [128, 4, 10]
    wf3 = W(15)[:, :, 0].reshape(4, 128, 10).transpose(1, 0, 2)
    ws[0:128, WFC3_OFF : WFC3_OFF + 40] = wf3.reshape(128, 40)
    d["wsmall"] = ws.astype(np.float16)

    bb = np.zeros((128, BIAS_COLS), np.float32)
    for j in range(13):
        co = LAYERS[j][1]
        t = max(1, co // 128)
        bb[: min(co, 128), _BCOLS[j] : _BCOLS[j] + t] = B(j).reshape(t, -1).T
    bb[:, BFC1_COL : BFC1_COL + 4] = B(13).reshape(4, 128).T
    bb[:, BFC2_COL : BFC2_COL + 4] = B(14).reshape(4, 128).T
    d["wbias"] = bb
    d["bfc3"] = B(15).reshape(1, 10).astype(np.float16)

    for j in range(4, 13):
        w = W(j)  # [cin, cout, 9]
        cin, cout = w.shape[0], w.shape[1]
        T = cin // 128
        blob = w.reshape(T, 128, cout, 9).transpose(1, 0, 3, 2)  # [128, T, 9, cout]
        d[f"w_l{j}"] = np.ascontiguousarray(blob.reshape(128, T * 9 * cout)).astype(np.float16)

    for k, j in [("wfc1", 13), ("wfc2", 14)]:
        w = W(j)[:, :, 0]  # [512, 512]
        d[k] = np.ascontiguousarray(
            w.reshape(4, 128, 512).transpose(1, 0, 2).reshape(128, 4 * 512)
        ).astype(np.float16)
    return d


def kernel(x, weights, biases):
    nc = build_nc()
    in_maps = [prep_core_inputs(c, x, weights, biases) for c in range(N_CORES)]
    res = run_bass_kernel_spmd(nc, in_maps, list(range(N_CORES)))
    out = np.zeros((MODEL_BS, IMG_BS, 10), np.float32)
    for c in range(N_CORES):
        m, h = c // 2, c % 2
        out[m, h * N : (h + 1) * N] = res.results[c]["y"]
    return out


# revision 11
# speedup vs baseline: 1.7342x; 1.2362x over previous
"""Batch-functional VGG16 forward on 8 Trainium2 NeuronCores.

Sharding: model_bs (4) x image-half (2) -> 8 cores. Each core runs the full
VGG forward for one model's weights on 16 of the 32 shared images.

Conv is 9-position implicit GEMM accumulated in PSUM (fp32r matmuls,
fp32 accumulate). Special packing for the small-Cin layers:
  - L0 (Cin=3):  dx-packed K=9 (3 shifted copies of x on partitions 0..8)
  - L1/L2 (Cin=64): position pairs packed into K=128 via a flat-shifted
    copy of the activations on partitions 64..127 (6 matmuls per chunk
    instead of 9).
Activations live in SBUF in zero-padded [P, (T), N, H+2, W+2] layout so a
conv tap (dy, dx) is just a base-offset shift of the rhs access pattern.
"""

import numpy as np
from contextlib import ExitStack

import concourse.bass as bass
import concourse.mybir as mybir
import concourse.tile as tile
from concourse.bass_utils import run_bass_kernel_spmd
from bass_rust import SyncInfo

f32 = mybir.dt.float32
f32r = mybir.dt.float16  # fp16 operands: pipelined LDWEIGHTS+FWL, half DMA
AF = mybir.ActivationFunctionType

N_CORES = 8
MODEL_BS, IMG_BS = 4, 32
N = 16  # images per core

# conv layers: (cin, cout, H_in, pool_after)
LAYERS = [
    (3, 64, 32, False),
    (64, 64, 32, True),
    (64, 128, 16, False),
    (128, 128, 16, True),
    (128, 256, 8, False),
    (256, 256, 8, False),
    (256, 256, 8, True),
    (256, 512, 4, False),
    (512, 512, 4, False),
    (512, 512, 4, True),
    (512, 512, 2, False),
    (512, 512, 2, False),
    (512, 512, 2, True),
]

# ---- wsmall blob column offsets (f32r, [128, WSMALL_COLS]) ----
W1_OFF = 0          # [9 rows, 3*64]   w1[c+3dx, dy*64+co]
W2P_OFF = 192       # [128, 3*64]      rows<64: w2[c,co,3dy]; rows>=64: 3dy+1
W2S_OFF = 384       # [64, 3*64]       w2[c,co,3dy+2]
W3P_OFF = 576       # [128, 3*128]
W3S_OFF = 960       # [64, 3*128]
W4_OFF = 1344       # [128, 9*128]
WFC3_OFF = 2496     # [128, 4*10]
WSMALL_COLS = 2536

# bias blob columns per layer (conv 0..12, then fc1, fc2)
_BCOLS = []
_c = 0
for (ci, co, h, p) in LAYERS:
    _BCOLS.append(_c)
    _c += max(1, co // 128)
BFC1_COL = _c
_c += 4
BFC2_COL = _c
_c += 4
BIAS_COLS = _c


def split_excess_waits(nc, max_regular=1, max_evsem=2):
    """This toolchain caps sync commands per instruction; spill extra waits
    onto preceding same-engine EventSemaphore ops."""
    uid = [0]
    for func in nc.m.functions:
        for block in func.blocks:
            new_insts = []
            for inst in block.instructions:
                si = inst.sync_info
                if si is not None and si.on_wait:
                    waits = list(si.on_wait)
                    cap = (
                        max_evsem
                        if isinstance(inst, mybir.InstEventSemaphore)
                        else max_regular
                    )
                    if len(waits) > cap:
                        keep, spill = waits[:cap], waits[cap:]
                        while spill:
                            batch, spill = spill[:max_evsem], spill[max_evsem:]
                            uid[0] += 1
                            new_insts.append(
                                mybir.InstEventSemaphore(
                                    name=f"waitspill_{uid[0]}",
                                    opcode="EventSemaphore",
                                    engine=inst.engine,
                                    sync_info=SyncInfo(on_wait=batch, on_update=[]),
                                    bass_nofuse=True,
                                )
                            )
                        inst.sync_info = SyncInfo(
                            on_wait=keep, on_update=list(si.on_update)
                        )
                new_insts.append(inst)
            block.instructions = new_insts
    return nc


def _flat(t):
    """Flat [P, free] view of a multi-dim SBUF tile."""
    shape = t.shape
    if len(shape) == 2:
        return t
    names = " ".join(f"d{i}" for i in range(1, len(shape)))
    return t.rearrange(f"p {names} -> p ({names})")


# timing-experiment modes: "full", "dma_only" (weight DMAs only),
# "static_weights" (matmuls read one resident dummy blob; no per-layer DMA)
MODE = "full"
# PREFIX: stop after this checkpoint id (0..7); 99 = full kernel
PREFIX = 99


class _EarlyStop(Exception):
    pass


def _ckpt(nc, a, small, cid):
    if PREFIX <= cid:
        outsb = small.tile([16, 10], f32, tag="outsb", name="outsb_early")
        nc.gpsimd.memset(outsb[:], 0.0)
        nc.sync.dma_start(a["y"], outsb[:])
        raise _EarlyStop


def emit(nc, tc, ctx, a):
    """Emit the whole per-core forward. `a` = dict of dram APs."""
    wbig = ctx.enter_context(tc.tile_pool(name="wbig", bufs=4))
    small = ctx.enter_context(tc.tile_pool(name="small", bufs=1))
    abuf = ctx.enter_context(tc.tile_pool(name="abuf", bufs=4))
    ptmp_pool = ctx.enter_context(tc.tile_pool(name="ptmp", bufs=2))
    ps = ctx.enter_context(tc.tile_pool(name="ps", bufs=7, space="PSUM"))
    psfc = ctx.enter_context(tc.tile_pool(name="psfc", bufs=1, space="PSUM"))

    # ---- persistent small tensors ----
    wsmall = small.tile([128, WSMALL_COLS], f32r, tag="wsmall")
    nc.sync.dma_start(wsmall[:], a["wsmall"])
    bias = small.tile([128, BIAS_COLS], f32, tag="bias")
    nc.sync.dma_start(bias[:], a["wbias"])
    bfc3 = small.tile([1, 10], f32r, tag="bfc3")
    nc.sync.dma_start(bfc3[:], a["bfc3"])
    ones = small.tile([1, 16], f32r, tag="ones")
    nc.gpsimd.memset(ones[:], 1.0)
    x3pair = small.tile([128, N, 18, 18], f32r, tag="x3pair")
    nc.gpsimd.memset(x3pair[:], 0.0)

    def relu_bias(dest_ap, psum_ap, layer_idx, tile_idx, parts):
        nc.scalar.activation(
            dest_ap,
            psum_ap,
            AF.Relu,
            bias=bias[0:parts, _BCOLS[layer_idx] + tile_idx : _BCOLS[layer_idx] + tile_idx + 1],
        )

    # ============ L0 + L1 + pool0: per-image software pipeline ============
    early = ctx.enter_context(tc.tile_pool(name="early", bufs=12))

    def stage_x(n):
        xp = early.tile([9, 34, 34], f32r, tag="early", name=f"xpack{n}")
        nc.gpsimd.memset(xp[:], 0.0)
        # three column-shifted copies straight from DRAM (dx = 0, 1, 2)
        nc.sync.dma_start(xp[0:3, 1:33, 1:33], a["x"][:, n, :, :])
        nc.sync.dma_start(xp[3:6, 1:33, 0:32], a["x"][:, n, :, :])
        nc.sync.dma_start(xp[6:9, 1:33, 0:31], a["x"][:, n, :, 1:32])
        return xp

    def conv_l0(n, xp):
        y1 = early.tile([128, 34, 34], f32r, tag="early", name=f"y1_{n}")
        nc.gpsimd.memset(y1[:], 0.0)
        for yh in range(2):
            y0 = yh * 16
            psum = ps.tile([64, 16, 32], f32, tag="ps", name=f"ps0_{n}_{yh}")
            for dy in range(3):
                nc.tensor.matmul(
                    psum[:],
                    wsmall[0:9, W1_OFF + dy * 64 : W1_OFF + (dy + 1) * 64],
                    xp[0:9, y0 + dy : y0 + dy + 16, 0:32],
                    start=(dy == 0),
                    stop=(dy == 2),
                )
            relu_bias(y1[0:64, 1 + y0 : 17 + y0, 1:33], psum[:], 0, 0, 64)
            # pair-pack copy: same values, one column left (flat +1)
            relu_bias(y1[64:128, 1 + y0 : 17 + y0, 0:32], psum[:], 0, 0, 64)
        return y1

    def conv_l1(n, y1):
        y2 = early.tile([64, 32, 32], f32r, tag="early", name=f"y2_{n}")
        for yh in range(2):
            y0 = yh * 16
            psum = ps.tile([64, 16, 32], f32, tag="ps", name=f"ps1_{n}_{yh}")
            k = 0
            for dy in range(3):
                nc.tensor.matmul(
                    psum[:],
                    wsmall[0:128, W2P_OFF + dy * 64 : W2P_OFF + (dy + 1) * 64],
                    y1[0:128, y0 + dy : y0 + dy + 16, 0:32],
                    start=(k == 0),
                    stop=False,
                )
                k += 1
            for dy in range(3):
                nc.tensor.matmul(
                    psum[:],
                    wsmall[0:64, W2S_OFF + dy * 64 : W2S_OFF + (dy + 1) * 64],
                    y1[0:64, y0 + dy : y0 + dy + 16, 2:34],
                    start=False,
                    stop=(dy == 2),
                )
            relu_bias(y2[0:64, y0 : y0 + 16, :], psum[:], 1, 0, 64)
        return y2

    def pool0(n, y2):
        p0 = ptmp_pool.tile([64, 32, 16], f32r, tag="ptmp", name=f"p0_{n}")
        yv = y2.rearrange("p h (w2 two) -> p h w2 two", two=2)
        nc.vector.tensor_max(p0[:], yv[:, :, :, 0], yv[:, :, :, 1])
        pv = p0.rearrange("p (h2 two) w2 -> p h2 two w2", two=2)
        nc.vector.tensor_max(x3pair[0:64, n, 1:17, 1:17], pv[:, :, 0, :], pv[:, :, 1, :])
        nc.vector.tensor_max(x3pair[64:128, n, 1:17, 0:16], pv[:, :, 0, :], pv[:, :, 1, :])

    xps = {}
    y1s = {}
    for n in range(N + 1):
        if n < N:
            xps[n] = stage_x(n)
            y1s[n] = conv_l0(n, xps[n])
        if n >= 1:
            y2 = conv_l1(n - 1, y1s.pop(n - 1))
            pool0(n - 1, y2)

    _ckpt(nc, a, small, 0)

    # ========== L2 (pair, 64->128) and L3 (128->128), chunk-interleaved ==========
    def conv_l2(c):
        n0 = c * 2
        x4c = early.tile([128, 2, 18, 18], f32r, tag="x4", bufs=3, name=f"x4_{c}")
        nc.gpsimd.memset(x4c[:], 0.0)
        psum = ps.tile([128, 2, 16, 16], f32, tag="ps", name=f"ps2_{c}")
        k = 0
        for dy in range(3):
            nc.tensor.matmul(
                psum[:],
                wsmall[0:128, W3P_OFF + dy * 128 : W3P_OFF + (dy + 1) * 128],
                x3pair[0:128, n0 : n0 + 2, dy : dy + 16, 0:16],
                start=(k == 0),
                stop=False,
            )
            k += 1
        for dy in range(3):
            nc.tensor.matmul(
                psum[:],
                wsmall[0:64, W3S_OFF + dy * 128 : W3S_OFF + (dy + 1) * 128],
                x3pair[0:64, n0 : n0 + 2, dy : dy + 16, 2:18],
                start=False,
                stop=(dy == 2),
            )
        relu_bias(x4c[:, :, 1:17, 1:17], psum[:], 2, 0, 128)
        return x4c

    def conv_l3(c, x4c, x5):
        n0 = c * 2
        y4c = early.tile([128, 2, 16, 16], f32r, tag="y4", bufs=3, name=f"y4_{c}")
        psum = ps.tile([128, 2, 16, 16], f32, tag="ps", name=f"ps3_{c}")
        for pos in range(9):
            dy, dx = pos // 3, pos % 3
            nc.tensor.matmul(
                psum[:],
                wsmall[0:128, W4_OFF + pos * 128 : W4_OFF + (pos + 1) * 128],
                x4c[0:128, :, dy : dy + 16, dx : dx + 16],
                start=(pos == 0),
                stop=(pos == 8),
            )
        relu_bias(y4c[:, :, :, :], psum[:], 3, 0, 128)
        # pool1 for these 2 images, straight into x5 interior
        p1 = ptmp_pool.tile([128, 2, 16, 8], f32r, tag="ptmp", name=f"p1_{c}")
        yv = y4c.rearrange("p n h (w2 two) -> p n h w2 two", two=2)
        nc.vector.tensor_max(p1[:], yv[:, :, :, :, 0], yv[:, :, :, :, 1])
        pv = p1.rearrange("p n (h2 two) w2 -> p n h2 two w2", two=2)
        nc.vector.tensor_max(
            x5[:, n0 : n0 + 2, 1:9, 1:9], pv[:, :, :, 0, :], pv[:, :, :, 1, :]
        )

    x5 = abuf.tile([128, N, 10, 10], f32r, tag="abuf")
    nc.gpsimd.memset(x5[:], 0.0)
    x4cs = {}
    for c in range(9):
        if c < 8:
            x4cs[c] = conv_l2(c)
        if c >= 1:
            conv_l3(c - 1, x4cs.pop(c - 1), x5)

    static_w = None
    if MODE == "static_weights":
        static_w = small.tile([128, 9, 512], f32r, tag="static_w")
        nc.sync.dma_start(_flat(static_w), a["w_l8"][:, 0 : 9 * 512])

    _ckpt(nc, a, small, 2)

    def load_wbig(name, t, cout):
        if MODE == "static_weights":
            return static_w[:, :, 0:cout]
        wt = wbig.tile([128, 9, cout], f32r, tag="wbig")
        nc.sync.dma_start(_flat(wt), a[name][:, t * 9 * cout : (t + 1) * 9 * cout])
        return wt

    if MODE == "dma_only":
        # only the weight traffic: all big-layer blobs + fc, then a dummy out
        for idx, cout, T in [(4, 256, 1), (5, 256, 2), (6, 256, 2), (7, 512, 2),
                             (8, 512, 4), (9, 512, 4), (10, 512, 4), (11, 512, 4),
                             (12, 512, 4)]:
            for t in range(T):
                wt = wbig.tile([128, 9, cout], f32r, tag="wbig", name=f"dma_{idx}_{t}")
                nc.sync.dma_start(
                    _flat(wt), a[f"w_l{idx}"][:, t * 9 * cout : (t + 1) * 9 * cout]
                )
        for nm in ("wfc1", "wfc2"):
            wt = wbig.tile([128, 4, 512], f32r, tag="wbig", name=f"dma_{nm}")
            nc.sync.dma_start(_flat(wt), a[nm][:, :])
        outsb = small.tile([16, 10], f32, tag="outsb")
        nc.gpsimd.memset(outsb[:], 0.0)
        nc.sync.dma_start(a["y"], outsb[:])
        return

    def conv_mid(layer_idx, x_in, x_out, wname, chunks, Hs, pad_out, pool_spec=None):
        """Generic mid conv. x_in: [128, T_in, N?, Hp, Wp] padded (T_in may be 1
        -> no tile dim). chunks: list of (n0, cnt). Hs: output spatial."""
        cin, cout, H, _ = LAYERS[layer_idx]
        Tin, Tout = max(1, cin // 128), max(1, cout // 128)
        wts = [load_wbig(wname, t, cout) for t in range(Tin)]
        single_chunk = len(chunks) == 1
        if single_chunk:
            (n0, cnt) = chunks[0]
            psums = [
                ps.tile([128, cnt, H, H], f32, tag="ps", name=f"ps_l{layer_idx}_{to}")
                for to in range(Tout)
            ]
            nmm = Tin * 9
            for ti in range(Tin):
                for to in range(Tout):
                    for pos in range(9):
                        dy, dx = pos // 3, pos % 3
                        k = ti * 9 + pos
                        rhs = (
                            x_in[:, ti, n0 : n0 + cnt, dy : dy + H, dx : dx + H]
                            if Tin > 1 or len(x_in.shape) == 5
                            else x_in[:, n0 : n0 + cnt, dy : dy + H, dx : dx + H]
                        )
                        nc.tensor.matmul(
                            psums[to][:],
                            wts[ti][:, pos, to * 128 : (to + 1) * 128],
                            rhs,
                            start=(k == 0),
                            stop=(k == nmm - 1),
                        )
            for to in range(Tout):
                dest = (
                    x_out[:, to, n0 : n0 + cnt, 1 : 1 + Hs, 1 : 1 + Hs]
                    if pad_out
                    else x_out[:, to, n0 : n0 + cnt, :, :]
                )
                relu_bias(dest, psums[to][:], layer_idx, to, 128)
        else:
            for (n0, cnt) in chunks:
                for to in range(Tout):
                    psum = ps.tile([128, cnt, H, H], f32, tag="ps")
                    nmm = Tin * 9
                    for ti in range(Tin):
                        for pos in range(9):
                            dy, dx = pos // 3, pos % 3
                            k = ti * 9 + pos
                            rhs = (
                                x_in[:, ti, n0 : n0 + cnt, dy : dy + H, dx : dx + H]
                                if Tin > 1
                                else x_in[:, n0 : n0 + cnt, dy : dy + H, dx : dx + H]
                            )
                            nc.tensor.matmul(
                                psum[:],
                                wts[ti][:, pos, to * 128 : (to + 1) * 128],
                                rhs,
                                start=(k == 0),
                                stop=(k == nmm - 1),
                            )
                    dest = (
                        x_out[:, to, n0 : n0 + cnt, 1 : 1 + Hs, 1 : 1 + Hs]
                        if pad_out
                        else x_out[:, to, n0 : n0 + cnt, :, :]
                    )
                    relu_bias(dest, psum[:], layer_idx, to, 128)

    def pool_padded(y_in, x_out, T, H):
        """2x2 maxpool y_in [128, T, N, H, H] -> x_out [128, T, N, H/2+2, H/2+2] interior."""
        Hh = H // 2
        for t in range(T):
            pt = ptmp_pool.tile([128, N, H, Hh], f32r, tag="ptmp")
            yv = y_in.rearrange("p t n h (w2 two) -> p t n h w2 two", two=2)
            nc.vector.tensor_max(pt[:], yv[:, t, :, :, :, 0], yv[:, t, :, :, :, 1])
            pv = pt.rearrange("p n (h2 two) w2 -> p n h2 two w2", two=2)
            nc.vector.tensor_max(
                x_out[:, t, :, 1 : 1 + Hh, 1 : 1 + Hh],
                pv[:, :, :, 0, :],
                pv[:, :, :, 1, :],
            )

    # L4: 128->256 @8x8
    x6 = abuf.tile([128, 2, N, 10, 10], f32r, tag="abuf")
    nc.gpsimd.memset(x6[:], 0.0)
    conv_mid(4, x5, x6, "w_l4", [(0, 8), (8, 8)], 8, pad_out=True)
    # L5: 256->256
    x7 = abuf.tile([128, 2, N, 10, 10], f32r, tag="abuf")
    nc.gpsimd.memset(x7[:], 0.0)
    conv_mid(5, x6, x7, "w_l5", [(0, 8), (8, 8)], 8, pad_out=True)
    _ckpt(nc, a, small, 3)
    # L6: 256->256, then pool2
    y7 = abuf.tile([128, 2, N, 8, 8], f32r, tag="abuf")
    conv_mid(6, x7, y7, "w_l6", [(0, 8), (8, 8)], 8, pad_out=False)
    x8 = abuf.tile([128, 2, N, 6, 6], f32r, tag="abuf")
    nc.gpsimd.memset(x8[:], 0.0)
    pool_padded(y7, x8, 2, 8)

    _ckpt(nc, a, small, 4)
    # L7: 256->512 @4x4
    x9 = abuf.tile([128, 4, N, 6, 6], f32r, tag="abuf")
    nc.gpsimd.memset(x9[:], 0.0)
    conv_mid(7, x8, x9, "w_l7", [(0, 16)], 4, pad_out=True)
    # L8
    x10 = abuf.tile([128, 4, N, 6, 6], f32r, tag="abuf")
    nc.gpsimd.memset(x10[:], 0.0)
    conv_mid(8, x9, x10, "w_l8", [(0, 16)], 4, pad_out=True)
    # L9, then pool3
    y10 = abuf.tile([128, 4, N, 4, 4], f32r, tag="abuf")
    conv_mid(9, x10, y10, "w_l9", [(0, 16)], 4, pad_out=False)
    x11 = abuf.tile([128, 4, N, 4, 4], f32r, tag="abuf")
    nc.gpsimd.memset(x11[:], 0.0)
    pool_padded(y10, x11, 4, 4)

    _ckpt(nc, a, small, 5)
    # L10..L12 @2x2
    x12 = abuf.tile([128, 4, N, 4, 4], f32r, tag="abuf")
    nc.gpsimd.memset(x12[:], 0.0)
    conv_mid(10, x11, x12, "w_l10", [(0, 16)], 2, pad_out=True)
    x13 = abuf.tile([128, 4, N, 4, 4], f32r, tag="abuf")
    nc.gpsimd.memset(x13[:], 0.0)
    conv_mid(11, x12, x13, "w_l11", [(0, 16)], 2, pad_out=True)
    y13 = abuf.tile([128, 4, N, 2, 2], f32r, tag="abuf")
    conv_mid(12, x13, y13, "w_l12", [(0, 16)], 2, pad_out=False)

    # pool4 -> xfc [128, 4, 16]
    xfc = small.tile([128, 4, N], f32r, tag="xfc")
    for t in range(4):
        pt = ptmp_pool.tile([128, N, 2], f32r, tag="ptmp")
        nc.vector.tensor_max(pt[:], y13[:, t, :, :, 0], y13[:, t, :, :, 1])
        nc.vector.tensor_max(xfc[:, t, :], pt[:, :, 0], pt[:, :, 1])

    _ckpt(nc, a, small, 6)

    # FC1, FC2: out[dout, img]
    def fc_layer(x_in, wname, bias_col, out_tag):
        wt = wbig.tile([128, 4, 512], f32r, tag="wbig")
        nc.sync.dma_start(_flat(wt), a[wname][:, :])
        x_out = small.tile([128, 4, N], f32r, tag=out_tag)
        for to in range(4):
            psum = ps.tile([128, N], f32, tag="ps")
            for ti in range(4):
                nc.tensor.matmul(
                    psum[:],
                    wt[:, ti, to * 128 : (to + 1) * 128],
                    x_in[:, ti, :],
                    start=(ti == 0),
                    stop=(ti == 3),
                )
            nc.scalar.activation(
                x_out[:, to, :],
                psum[:],
                AF.Identity,
                bias=bias[:, bias_col + to : bias_col + to + 1],
            )
        return x_out

    xfc2 = fc_layer(xfc, "wfc1", BFC1_COL, "xfc2")
    xfc3 = fc_layer(xfc2, "wfc2", BFC2_COL, "xfc3")

    # FC3 (flipped): psum[img, dout] = sum_t xfc3[:,t,:].T @ wfc3_t + ones.T @ bfc3
    psum3 = psfc.tile([16, 10], f32, tag="psfc")
    for t in range(4):
        nc.tensor.matmul(
            psum3[:],
            xfc3[:, t, :],
            wsmall[0:128, WFC3_OFF + t * 10 : WFC3_OFF + (t + 1) * 10],
            start=(t == 0),
            stop=False,
        )
    nc.tensor.matmul(psum3[:], ones[0:1, 0:16], bfc3[0:1, 0:10], start=False, stop=True)
    outsb = small.tile([16, 10], f32, tag="outsb")
    nc.scalar.copy(outsb[:], psum3[:])
    nc.sync.dma_start(a["y"], outsb[:])


def build_nc(repeat=1):
    nc = bass.Bass("TRN2", target_bir_lowering=False, debug=False, num_devices=N_CORES)
    a = {}
    a["x"] = nc.dram_tensor("x", [3, N, 32, 32], f32r, kind="ExternalInput").ap()
    a["wsmall"] = nc.dram_tensor("wsmall", [128, WSMALL_COLS], f32r, kind="ExternalInput").ap()
    a["wbias"] = nc.dram_tensor("wbias", [128, BIAS_COLS], f32, kind="ExternalInput").ap()
    a["bfc3"] = nc.dram_tensor("bfc3", [1, 10], f32r, kind="ExternalInput").ap()
    for idx, cols in [(4, 1 * 9 * 256), (5, 2 * 9 * 256), (6, 2 * 9 * 256),
                      (7, 2 * 9 * 512), (8, 4 * 9 * 512), (9, 4 * 9 * 512),
                      (10, 4 * 9 * 512), (11, 4 * 9 * 512), (12, 4 * 9 * 512)]:
        a[f"w_l{idx}"] = nc.dram_tensor(f"w_l{idx}", [128, cols], f32r, kind="ExternalInput").ap()
    a["wfc1"] = nc.dram_tensor("wfc1", [128, 4 * 512], f32r, kind="ExternalInput").ap()
    a["wfc2"] = nc.dram_tensor("wfc2", [128, 4 * 512], f32r, kind="ExternalInput").ap()
    a["y"] = nc.dram_tensor("y", [N, 10], f32, kind="ExternalOutput").ap()

    with tile.TileContext(nc) as tc:
        with ExitStack() as ctx:
            def emit_safe():
                try:
                    emit(nc, tc, ctx, a)
                except _EarlyStop:
                    pass

            if repeat > 1:
                with tc.For_i(0, repeat, 1):
                    emit_safe()
            else:
                emit_safe()
    split_excess_waits(nc)
    return nc


def prep_core_inputs(core, x, weights, biases):
    m = core // 2
    h = core % 2
    d = {}
    d["x"] = np.ascontiguousarray(
        np.asarray(x[h * N : (h + 1) * N]).transpose(1, 0, 2, 3)
    ).astype(np.float16)

    def W(j):
        return np.asarray(weights[j][m]).astype(np.float32)

    def B(j):
        return np.asarray(biases[j][m]).astype(np.float32)[:, 0]

    ws = np.zeros((128, WSMALL_COLS), np.float32)
    # L0: [9, 3, 64]: row c+3dx, col dy*64+co  <- w1[c, co, 3dy+dx]
    w1 = W(0).reshape(3, 64, 3, 3)  # [c, co, dy, dx]
    ws[0:9, W1_OFF : W1_OFF + 192] = (
        w1.transpose(3, 0, 2, 1).reshape(9, 192)
    )
    # L1 pair/single
    w2 = W(1).reshape(64, 64, 3, 3)
    pair = np.concatenate([w2[:, :, :, 0], w2[:, :, :, 1]], axis=0)  # [128, co, dy]
    ws[0:128, W2P_OFF : W2P_OFF + 192] = pair.transpose(0, 2, 1).reshape(128, 192)
    ws[0:64, W2S_OFF : W2S_OFF + 192] = (
        w2[:, :, :, 2].transpose(0, 2, 1).reshape(64, 192)
    )
    # L2 pair/single (cout=128)
    w3 = W(2).reshape(64, 128, 3, 3)
    pair = np.concatenate([w3[:, :, :, 0], w3[:, :, :, 1]], axis=0)
    ws[0:128, W3P_OFF : W3P_OFF + 384] = pair.transpose(0, 2, 1).reshape(128, 384)
    ws[0:64, W3S_OFF : W3S_OFF + 384] = (
        w3[:, :, :, 2].transpose(0, 2, 1).reshape(64, 384)
    )
    # L3: [128, 9*128]
    ws[0:128, W4_OFF : W4_OFF + 1152] = W(3).transpose(0, 2, 1).reshape(128, 1152)
    # FC3 weights [512, 10, 1] -> [128, 4, 10]
    wf3 = W(15)[:, :, 0].reshape(4, 128, 10).transpose(1, 0, 2)
    ws[0:128, WFC3_OFF : WFC3_OFF + 40] = wf3.reshape(128, 40)
    d["wsmall"] = ws.astype(np.float16)

    bb = np.zeros((128, BIAS_COLS), np.float32)
    for j in range(13):
        co = LAYERS[j][1]
        t = max(1, co // 128)
        bb[: min(co, 128), _BCOLS[j] : _BCOLS[j] + t] = B(j).reshape(t, -1).T
    bb[:, BFC1_COL : BFC1_COL + 4] = B(13).reshape(4, 128).T
    bb[:, BFC2_COL : BFC2_COL + 4] = B(14).reshape(4, 128).T
    d["wbias"] = bb
    d["bfc3"] = B(15).reshape(1, 10).astype(np.float16)

    for j in range(4, 13):
        w = W(j)  # [cin, cout, 9]
        cin, cout = w.shape[0], w.shape[1]
        T = cin // 128
        blob = w.reshape(T, 128, cout, 9).transpose(1, 0, 3, 2)  # [128, T, 9, cout]
        d[f"w_l{j}"] = np.ascontiguousarray(blob.reshape(128, T * 9 * cout)).astype(np.float16)

    for k, j in [("wfc1", 13), ("wfc2", 14)]:
        w = W(j)[:, :, 0]  # [512, 512]
        d[k] = np.ascontiguousarray(
            w.reshape(4, 128, 512).transpose(1, 0, 2).reshape(128, 4 * 512)
        ).astype(np.float16)
    return d


def kernel(x, weights, biases):
    nc = build_nc()
    in_maps = [prep_core_inputs(c, x, weights, biases) for c in range(N_CORES)]
    res = run_bass_kernel_spmd(nc, in_maps, list(range(N_CORES)))
    out = np.zeros((MODEL_BS, IMG_BS, 10), np.float32)
    for c in range(N_CORES):
        m, h = c // 2, c % 2
        out[m, h * N : (h + 1) * N] = res.results[c]["y"]
    return out


# revision 12
# speedup vs baseline: 1.7534x; 1.0111x over previous
"""Batch-functional VGG16 forward on 8 Trainium2 NeuronCores.

Sharding: model_bs (4) x image-half (2) -> 8 cores. Each core runs the full
VGG forward for one model's weights on 16 of the 32 shared images.

Conv is 9-position implicit GEMM accumulated in PSUM (fp32r matmuls,
fp32 accumulate). Special packing for the small-Cin layers:
  - L0 (Cin=3):  dx-packed K=9 (3 shifted copies of x on partitions 0..8)
  - L1/L2 (Cin=64): position pairs packed into K=128 via a flat-shifted
    copy of the activations on partitions 64..127 (6 matmuls per chunk
    instead of 9).
Activations live in SBUF in zero-padded [P, (T), N, H+2, W+2] layout so a
conv tap (dy, dx) is just a base-offset shift of the rhs access pattern.
"""

import numpy as np
from contextlib import ExitStack

import concourse.bass as bass
import concourse.mybir as mybir
import concourse.tile as tile
from concourse.bass_utils import run_bass_kernel_spmd
from bass_rust import SyncInfo

f32 = mybir.dt.float32
f32r = mybir.dt.float16  # fp16 operands: pipelined LDWEIGHTS+FWL, half DMA
AF = mybir.ActivationFunctionType

N_CORES = 8
MODEL_BS, IMG_BS = 4, 32
N = 16  # images per core

# conv layers: (cin, cout, H_in, pool_after)
LAYERS = [
    (3, 64, 32, False),
    (64, 64, 32, True),
    (64, 128, 16, False),
    (128, 128, 16, True),
    (128, 256, 8, False),
    (256, 256, 8, False),
    (256, 256, 8, True),
    (256, 512, 4, False),
    (512, 512, 4, False),
    (512, 512, 4, True),
    (512, 512, 2, False),
    (512, 512, 2, False),
    (512, 512, 2, True),
]

# ---- wsmall blob column offsets (f32r, [128, WSMALL_COLS]) ----
W1_OFF = 0          # [9 rows, 3*64]   w1[c+3dx, dy*64+co]
W2P_OFF = 192       # [128, 3*64]      rows<64: w2[c,co,3dy]; rows>=64: 3dy+1
W2S_OFF = 384       # [64, 3*64]       w2[c,co,3dy+2]
W3P_OFF = 576       # [128, 3*128]
W3S_OFF = 960       # [64, 3*128]
W4_OFF = 1344       # [128, 9*128]
WFC3_OFF = 2496     # [128, 4*10]
WSMALL_COLS = 2536

# bias blob columns per layer (conv 0..12, then fc1, fc2)
_BCOLS = []
_c = 0
for (ci, co, h, p) in LAYERS:
    _BCOLS.append(_c)
    _c += max(1, co // 128)
BFC1_COL = _c
_c += 4
BFC2_COL = _c
_c += 4
BIAS_COLS = _c


def split_excess_waits(nc, max_regular=1, max_evsem=2):
    """This toolchain caps sync commands per instruction; spill extra waits
    onto preceding same-engine EventSemaphore ops."""
    uid = [0]
    for func in nc.m.functions:
        for block in func.blocks:
            new_insts = []
            for inst in block.instructions:
                si = inst.sync_info
                if si is not None and si.on_wait:
                    waits = list(si.on_wait)
                    cap = (
                        max_evsem
                        if isinstance(inst, mybir.InstEventSemaphore)
                        else max_regular
                    )
                    if len(waits) > cap:
                        keep, spill = waits[:cap], waits[cap:]
                        while spill:
                            batch, spill = spill[:max_evsem], spill[max_evsem:]
                            uid[0] += 1
                            new_insts.append(
                                mybir.InstEventSemaphore(
                                    name=f"waitspill_{uid[0]}",
                                    opcode="EventSemaphore",
                                    engine=inst.engine,
                                    sync_info=SyncInfo(on_wait=batch, on_update=[]),
                                    bass_nofuse=True,
                                )
                            )
                        inst.sync_info = SyncInfo(
                            on_wait=keep, on_update=list(si.on_update)
                        )
                new_insts.append(inst)
            block.instructions = new_insts
    return nc


def _flat(t):
    """Flat [P, free] view of a multi-dim SBUF tile."""
    shape = t.shape
    if len(shape) == 2:
        return t
    names = " ".join(f"d{i}" for i in range(1, len(shape)))
    return t.rearrange(f"p {names} -> p ({names})")


# timing-experiment modes: "full", "dma_only" (weight DMAs only),
# "static_weights" (matmuls read one resident dummy blob; no per-layer DMA)
MODE = "full"
# PREFIX: stop after this checkpoint id (0..7); 99 = full kernel
PREFIX = 99


class _EarlyStop(Exception):
    pass


def _ckpt(nc, a, small, cid):
    if PREFIX <= cid:
        outsb = small.tile([16, 10], f32, tag="outsb", name="outsb_early")
        nc.gpsimd.memset(outsb[:], 0.0)
        nc.sync.dma_start(a["y"], outsb[:])
        raise _EarlyStop


def emit(nc, tc, ctx, a):
    """Emit the whole per-core forward. `a` = dict of dram APs."""
    wbig = ctx.enter_context(tc.tile_pool(name="wbig", bufs=4))
    small = ctx.enter_context(tc.tile_pool(name="small", bufs=1))
    abuf = ctx.enter_context(tc.tile_pool(name="abuf", bufs=4))
    ptmp_pool = ctx.enter_context(tc.tile_pool(name="ptmp", bufs=2))
    ps = ctx.enter_context(tc.tile_pool(name="ps", bufs=7, space="PSUM"))
    psfc = ctx.enter_context(tc.tile_pool(name="psfc", bufs=1, space="PSUM"))

    # ---- persistent small tensors ----
    wsmall = small.tile([128, WSMALL_COLS], f32r, tag="wsmall")
    nc.sync.dma_start(wsmall[:], a["wsmall"])
    bias = small.tile([128, BIAS_COLS], f32, tag="bias")
    nc.sync.dma_start(bias[:], a["wbias"])
    bfc3 = small.tile([1, 10], f32r, tag="bfc3")
    nc.sync.dma_start(bfc3[:], a["bfc3"])
    ones = small.tile([1, 16], f32r, tag="ones")
    nc.gpsimd.memset(ones[:], 1.0)
    x3pair = small.tile([128, N, 18, 18], f32r, tag="x3pair")
    nc.gpsimd.memset(x3pair[:], 0.0)

    def relu_bias(dest_ap, psum_ap, layer_idx, tile_idx, parts):
        nc.scalar.activation(
            dest_ap,
            psum_ap,
            AF.Relu,
            bias=bias[0:parts, _BCOLS[layer_idx] + tile_idx : _BCOLS[layer_idx] + tile_idx + 1],
        )

    # ============ L0 + L1 + pool0: per-image software pipeline ============
    early = ctx.enter_context(tc.tile_pool(name="early", bufs=12))

    def stage_x(n):
        xp = early.tile([9, 34, 34], f32r, tag="early", name=f"xpack{n}")
        nc.gpsimd.memset(xp[:], 0.0)
        # three column-shifted copies straight from DRAM (dx = 0, 1, 2)
        nc.sync.dma_start(xp[0:3, 1:33, 1:33], a["x"][:, n, :, :])
        nc.sync.dma_start(xp[3:6, 1:33, 0:32], a["x"][:, n, :, :])
        nc.sync.dma_start(xp[6:9, 1:33, 0:31], a["x"][:, n, :, 1:32])
        return xp

    def conv_l0(n, xp):
        y1 = early.tile([128, 34, 34], f32r, tag="early", name=f"y1_{n}")
        nc.gpsimd.memset(y1[:], 0.0)
        for yh in range(2):
            y0 = yh * 16
            psum = ps.tile([64, 16, 32], f32, tag="ps", name=f"ps0_{n}_{yh}")
            for dy in range(3):
                nc.tensor.matmul(
                    psum[:],
                    wsmall[0:9, W1_OFF + dy * 64 : W1_OFF + (dy + 1) * 64],
                    xp[0:9, y0 + dy : y0 + dy + 16, 0:32],
                    start=(dy == 0),
                    stop=(dy == 2),
                )
            relu_bias(y1[0:64, 1 + y0 : 17 + y0, 1:33], psum[:], 0, 0, 64)
            # pair-pack copy: same values, one column left (flat +1)
            relu_bias(y1[64:128, 1 + y0 : 17 + y0, 0:32], psum[:], 0, 0, 64)
        return y1

    def conv_l1(n, y1):
        y2 = early.tile([64, 32, 32], f32r, tag="early", name=f"y2_{n}")
        for yh in range(2):
            y0 = yh * 16
            psum = ps.tile([64, 16, 32], f32, tag="ps", name=f"ps1_{n}_{yh}")
            k = 0
            for dy in range(3):
                nc.tensor.matmul(
                    psum[:],
                    wsmall[0:128, W2P_OFF + dy * 64 : W2P_OFF + (dy + 1) * 64],
                    y1[0:128, y0 + dy : y0 + dy + 16, 0:32],
                    start=(k == 0),
                    stop=False,
                )
                k += 1
            for dy in range(3):
                nc.tensor.matmul(
                    psum[:],
                    wsmall[0:64, W2S_OFF + dy * 64 : W2S_OFF + (dy + 1) * 64],
                    y1[0:64, y0 + dy : y0 + dy + 16, 2:34],
                    start=False,
                    stop=(dy == 2),
                )
            relu_bias(y2[0:64, y0 : y0 + 16, :], psum[:], 1, 0, 64)
        return y2

    def pool0(n, y2):
        p0 = ptmp_pool.tile([64, 32, 16], f32r, tag="ptmp", name=f"p0_{n}")
        yv = y2.rearrange("p h (w2 two) -> p h w2 two", two=2)
        nc.vector.tensor_max(p0[:], yv[:, :, :, 0], yv[:, :, :, 1])
        pv = p0.rearrange("p (h2 two) w2 -> p h2 two w2", two=2)
        nc.vector.tensor_max(x3pair[0:64, n, 1:17, 1:17], pv[:, :, 0, :], pv[:, :, 1, :])
        nc.vector.tensor_max(x3pair[64:128, n, 1:17, 0:16], pv[:, :, 0, :], pv[:, :, 1, :])

    xps = {}
    y1s = {}
    for n in range(N + 1):
        if n < N:
            xps[n] = stage_x(n)
            y1s[n] = conv_l0(n, xps[n])
        if n >= 1:
            y2 = conv_l1(n - 1, y1s.pop(n - 1))
            pool0(n - 1, y2)

    _ckpt(nc, a, small, 0)

    # ========== L2 (pair, 64->128) and L3 (128->128), chunk-interleaved ==========
    def conv_l2(c):
        n0 = c * 2
        x4c = early.tile([128, 2, 18, 18], f32r, tag="x4", bufs=3, name=f"x4_{c}")
        nc.gpsimd.memset(x4c[:], 0.0)
        psum = ps.tile([128, 2, 16, 16], f32, tag="ps", name=f"ps2_{c}")
        k = 0
        for dy in range(3):
            nc.tensor.matmul(
                psum[:],
                wsmall[0:128, W3P_OFF + dy * 128 : W3P_OFF + (dy + 1) * 128],
                x3pair[0:128, n0 : n0 + 2, dy : dy + 16, 0:16],
                start=(k == 0),
                stop=False,
            )
            k += 1
        for dy in range(3):
            nc.tensor.matmul(
                psum[:],
                wsmall[0:64, W3S_OFF + dy * 128 : W3S_OFF + (dy + 1) * 128],
                x3pair[0:64, n0 : n0 + 2, dy : dy + 16, 2:18],
                start=False,
                stop=(dy == 2),
            )
        relu_bias(x4c[:, :, 1:17, 1:17], psum[:], 2, 0, 128)
        return x4c

    def conv_l3(c, x4c, x5):
        n0 = c * 2
        y4c = early.tile([128, 2, 16, 16], f32r, tag="y4", bufs=3, name=f"y4_{c}")
        psum = ps.tile([128, 2, 16, 16], f32, tag="ps", name=f"ps3_{c}")
        for pos in range(9):
            dy, dx = pos // 3, pos % 3
            nc.tensor.matmul(
                psum[:],
                wsmall[0:128, W4_OFF + pos * 128 : W4_OFF + (pos + 1) * 128],
                x4c[0:128, :, dy : dy + 16, dx : dx + 16],
                start=(pos == 0),
                stop=(pos == 8),
            )
        relu_bias(y4c[:, :, :, :], psum[:], 3, 0, 128)
        # pool1 for these 2 images, straight into x5 interior
        p1 = ptmp_pool.tile([128, 2, 16, 8], f32r, tag="ptmp", name=f"p1_{c}")
        yv = y4c.rearrange("p n h (w2 two) -> p n h w2 two", two=2)
        nc.vector.tensor_max(p1[:], yv[:, :, :, :, 0], yv[:, :, :, :, 1])
        pv = p1.rearrange("p n (h2 two) w2 -> p n h2 two w2", two=2)
        nc.vector.tensor_max(
            x5[:, n0 : n0 + 2, 1:9, 1:9], pv[:, :, :, 0, :], pv[:, :, :, 1, :]
        )

    x5 = abuf.tile([128, N, 10, 10], f32r, tag="abuf")
    nc.gpsimd.memset(x5[:], 0.0)
    x4cs = {}
    for c in range(9):
        if c < 8:
            x4cs[c] = conv_l2(c)
        if c >= 1:
            conv_l3(c - 1, x4cs.pop(c - 1), x5)

    static_w = None
    if MODE == "static_weights":
        static_w = small.tile([128, 9, 512], f32r, tag="static_w")
        nc.sync.dma_start(_flat(static_w), a["w_l8_0"])

    _ckpt(nc, a, small, 2)

    def load_wbig(name, t, cout):
        if MODE == "static_weights":
            return static_w[:, :, 0:cout]
        wt = wbig.tile([128, 9, cout], f32r, tag="wbig")
        nc.sync.dma_start(_flat(wt), a[f"{name}_{t}"])
        return wt

    if MODE == "dma_only":
        # only the weight traffic: all big-layer blobs + fc, then a dummy out
        for idx, cout, T in [(4, 256, 1), (5, 256, 2), (6, 256, 2), (7, 512, 2),
                             (8, 512, 4), (9, 512, 4), (10, 512, 4), (11, 512, 4),
                             (12, 512, 4)]:
            for t in range(T):
                wt = wbig.tile([128, 9, cout], f32r, tag="wbig", name=f"dma_{idx}_{t}")
                nc.sync.dma_start(_flat(wt), a[f"w_l{idx}_{t}"])
        for nm in ("wfc1", "wfc2"):
            wt = wbig.tile([128, 4, 512], f32r, tag="wbig", name=f"dma_{nm}")
            nc.sync.dma_start(_flat(wt), a[nm][:, :])
        outsb = small.tile([16, 10], f32, tag="outsb")
        nc.gpsimd.memset(outsb[:], 0.0)
        nc.sync.dma_start(a["y"], outsb[:])
        return

    def conv_mid(layer_idx, x_in, x_out, wname, chunks, Hs, pad_out, pool_spec=None):
        """Generic mid conv. x_in: [128, T_in, N?, Hp, Wp] padded (T_in may be 1
        -> no tile dim). chunks: list of (n0, cnt). Hs: output spatial."""
        cin, cout, H, _ = LAYERS[layer_idx]
        Tin, Tout = max(1, cin // 128), max(1, cout // 128)
        wts = [load_wbig(wname, t, cout) for t in range(Tin)]
        single_chunk = len(chunks) == 1
        if single_chunk:
            (n0, cnt) = chunks[0]
            psums = [
                ps.tile([128, cnt, H, H], f32, tag="ps", name=f"ps_l{layer_idx}_{to}")
                for to in range(Tout)
            ]
            nmm = Tin * 9
            for ti in range(Tin):
                for to in range(Tout):
                    for pos in range(9):
                        dy, dx = pos // 3, pos % 3
                        k = ti * 9 + pos
                        rhs = (
                            x_in[:, ti, n0 : n0 + cnt, dy : dy + H, dx : dx + H]
                            if Tin > 1 or len(x_in.shape) == 5
                            else x_in[:, n0 : n0 + cnt, dy : dy + H, dx : dx + H]
                        )
                        nc.tensor.matmul(
                            psums[to][:],
                            wts[ti][:, pos, to * 128 : (to + 1) * 128],
                            rhs,
                            start=(k == 0),
                            stop=(k == nmm - 1),
                        )
            for to in range(Tout):
                dest = (
                    x_out[:, to, n0 : n0 + cnt, 1 : 1 + Hs, 1 : 1 + Hs]
                    if pad_out
                    else x_out[:, to, n0 : n0 + cnt, :, :]
                )
                relu_bias(dest, psums[to][:], layer_idx, to, 128)
        else:
            for (n0, cnt) in chunks:
                for to in range(Tout):
                    psum = ps.tile([128, cnt, H, H], f32, tag="ps")
                    nmm = Tin * 9
                    for ti in range(Tin):
                        for pos in range(9):
                            dy, dx = pos // 3, pos % 3
                            k = ti * 9 + pos
                            rhs = (
                                x_in[:, ti, n0 : n0 + cnt, dy : dy + H, dx : dx + H]
                                if Tin > 1
                                else x_in[:, n0 : n0 + cnt, dy : dy + H, dx : dx + H]
                            )
                            nc.tensor.matmul(
                                psum[:],
                                wts[ti][:, pos, to * 128 : (to + 1) * 128],
                                rhs,
                                start=(k == 0),
                                stop=(k == nmm - 1),
                            )
                    dest = (
                        x_out[:, to, n0 : n0 + cnt, 1 : 1 + Hs, 1 : 1 + Hs]
                        if pad_out
                        else x_out[:, to, n0 : n0 + cnt, :, :]
                    )
                    relu_bias(dest, psum[:], layer_idx, to, 128)

    def pool_padded(y_in, x_out, T, H):
        """2x2 maxpool y_in [128, T, N, H, H] -> x_out [128, T, N, H/2+2, H/2+2] interior."""
        Hh = H // 2
        for t in range(T):
            pt = ptmp_pool.tile([128, N, H, Hh], f32r, tag="ptmp")
            yv = y_in.rearrange("p t n h (w2 two) -> p t n h w2 two", two=2)
            nc.vector.tensor_max(pt[:], yv[:, t, :, :, :, 0], yv[:, t, :, :, :, 1])
            pv = pt.rearrange("p n (h2 two) w2 -> p n h2 two w2", two=2)
            nc.vector.tensor_max(
                x_out[:, t, :, 1 : 1 + Hh, 1 : 1 + Hh],
                pv[:, :, :, 0, :],
                pv[:, :, :, 1, :],
            )

    # L4: 128->256 @8x8
    x6 = abuf.tile([128, 2, N, 10, 10], f32r, tag="abuf")
    nc.gpsimd.memset(x6[:], 0.0)
    conv_mid(4, x5, x6, "w_l4", [(0, 8), (8, 8)], 8, pad_out=True)
    # L5: 256->256
    x7 = abuf.tile([128, 2, N, 10, 10], f32r, tag="abuf")
    nc.gpsimd.memset(x7[:], 0.0)
    conv_mid(5, x6, x7, "w_l5", [(0, 8), (8, 8)], 8, pad_out=True)
    _ckpt(nc, a, small, 3)
    # L6: 256->256, then pool2
    y7 = abuf.tile([128, 2, N, 8, 8], f32r, tag="abuf")
    conv_mid(6, x7, y7, "w_l6", [(0, 8), (8, 8)], 8, pad_out=False)
    x8 = abuf.tile([128, 2, N, 6, 6], f32r, tag="abuf")
    nc.gpsimd.memset(x8[:], 0.0)
    pool_padded(y7, x8, 2, 8)

    _ckpt(nc, a, small, 4)
    # L7: 256->512 @4x4
    x9 = abuf.tile([128, 4, N, 6, 6], f32r, tag="abuf")
    nc.gpsimd.memset(x9[:], 0.0)
    conv_mid(7, x8, x9, "w_l7", [(0, 16)], 4, pad_out=True)
    # L8
    x10 = abuf.tile([128, 4, N, 6, 6], f32r, tag="abuf")
    nc.gpsimd.memset(x10[:], 0.0)
    conv_mid(8, x9, x10, "w_l8", [(0, 16)], 4, pad_out=True)
    # L9, then pool3
    y10 = abuf.tile([128, 4, N, 4, 4], f32r, tag="abuf")
    conv_mid(9, x10, y10, "w_l9", [(0, 16)], 4, pad_out=False)
    x11 = abuf.tile([128, 4, N, 4, 4], f32r, tag="abuf")
    nc.gpsimd.memset(x11[:], 0.0)
    pool_padded(y10, x11, 4, 4)

    _ckpt(nc, a, small, 5)
    # L10..L12 @2x2
    x12 = abuf.tile([128, 4, N, 4, 4], f32r, tag="abuf")
    nc.gpsimd.memset(x12[:], 0.0)
    conv_mid(10, x11, x12, "w_l10", [(0, 16)], 2, pad_out=True)
    x13 = abuf.tile([128, 4, N, 4, 4], f32r, tag="abuf")
    nc.gpsimd.memset(x13[:], 0.0)
    conv_mid(11, x12, x13, "w_l11", [(0, 16)], 2, pad_out=True)
    y13 = abuf.tile([128, 4, N, 2, 2], f32r, tag="abuf")
    conv_mid(12, x13, y13, "w_l12", [(0, 16)], 2, pad_out=False)

    # pool4 -> xfc [128, 4, 16]
    xfc = small.tile([128, 4, N], f32r, tag="xfc")
    for t in range(4):
        pt = ptmp_pool.tile([128, N, 2], f32r, tag="ptmp")
        nc.vector.tensor_max(pt[:], y13[:, t, :, :, 0], y13[:, t, :, :, 1])
        nc.vector.tensor_max(xfc[:, t, :], pt[:, :, 0], pt[:, :, 1])

    _ckpt(nc, a, small, 6)

    # FC1, FC2: out[dout, img]
    def fc_layer(x_in, wname, bias_col, out_tag):
        wt = wbig.tile([128, 4, 512], f32r, tag="wbig")
        nc.sync.dma_start(_flat(wt), a[wname][:, :])
        x_out = small.tile([128, 4, N], f32r, tag=out_tag)
        for to in range(4):
            psum = ps.tile([128, N], f32, tag="ps")
            for ti in range(4):
                nc.tensor.matmul(
                    psum[:],
                    wt[:, ti, to * 128 : (to + 1) * 128],
                    x_in[:, ti, :],
                    start=(ti == 0),
                    stop=(ti == 3),
                )
            nc.scalar.activation(
                x_out[:, to, :],
                psum[:],
                AF.Identity,
                bias=bias[:, bias_col + to : bias_col + to + 1],
            )
        return x_out

    xfc2 = fc_layer(xfc, "wfc1", BFC1_COL, "xfc2")
    xfc3 = fc_layer(xfc2, "wfc2", BFC2_COL, "xfc3")

    # FC3 (flipped): psum[img, dout] = sum_t xfc3[:,t,:].T @ wfc3_t + ones.T @ bfc3
    psum3 = psfc.tile([16, 10], f32, tag="psfc")
    for t in range(4):
        nc.tensor.matmul(
            psum3[:],
            xfc3[:, t, :],
            wsmall[0:128, WFC3_OFF + t * 10 : WFC3_OFF + (t + 1) * 10],
            start=(t == 0),
            stop=False,
        )
    nc.tensor.matmul(psum3[:], ones[0:1, 0:16], bfc3[0:1, 0:10], start=False, stop=True)
    outsb = small.tile([16, 10], f32, tag="outsb")
    nc.scalar.copy(outsb[:], psum3[:])
    nc.sync.dma_start(a["y"], outsb[:])


def build_nc(repeat=1):
    nc = bass.Bass("TRN2", target_bir_lowering=False, debug=False, num_devices=N_CORES)
    a = {}
    a["x"] = nc.dram_tensor("x", [3, N, 32, 32], f32r, kind="ExternalInput").ap()
    a["wsmall"] = nc.dram_tensor("wsmall", [128, WSMALL_COLS], f32r, kind="ExternalInput").ap()
    a["wbias"] = nc.dram_tensor("wbias", [128, BIAS_COLS], f32, kind="ExternalInput").ap()
    a["bfc3"] = nc.dram_tensor("bfc3", [1, 10], f32r, kind="ExternalInput").ap()
    for idx, cout, T in [(4, 256, 1), (5, 256, 2), (6, 256, 2), (7, 512, 2),
                         (8, 512, 4), (9, 512, 4), (10, 512, 4), (11, 512, 4),
                         (12, 512, 4)]:
        for t in range(T):
            a[f"w_l{idx}_{t}"] = nc.dram_tensor(
                f"w_l{idx}_{t}", [128, 9 * cout], f32r, kind="ExternalInput"
            ).ap()
    a["wfc1"] = nc.dram_tensor("wfc1", [128, 4 * 512], f32r, kind="ExternalInput").ap()
    a["wfc2"] = nc.dram_tensor("wfc2", [128, 4 * 512], f32r, kind="ExternalInput").ap()
    a["y"] = nc.dram_tensor("y", [N, 10], f32, kind="ExternalOutput").ap()

    with tile.TileContext(nc) as tc:
        with ExitStack() as ctx:
            def emit_safe():
                try:
                    emit(nc, tc, ctx, a)
                except _EarlyStop:
                    pass

            if repeat > 1:
                with tc.For_i(0, repeat, 1):
                    emit_safe()
            else:
                emit_safe()
    split_excess_waits(nc)
    return nc


def prep_core_inputs(core, x, weights, biases):
    m = core // 2
    h = core % 2
    d = {}
    d["x"] = np.ascontiguousarray(
        np.asarray(x[h * N : (h + 1) * N]).transpose(1, 0, 2, 3)
    ).astype(np.float16)

    def W(j):
        return np.asarray(weights[j][m]).astype(np.float32)

    def B(j):
        return np.asarray(biases[j][m]).astype(np.float32)[:, 0]

    ws = np.zeros((128, WSMALL_COLS), np.float32)
    # L0: [9, 3, 64]: row c+3dx, col dy*64+co  <- w1[c, co, 3dy+dx]
    w1 = W(0).reshape(3, 64, 3, 3)  # [c, co, dy, dx]
    ws[0:9, W1_OFF : W1_OFF + 192] = (
        w1.transpose(3, 0, 2, 1).reshape(9, 192)
    )
    # L1 pair/single
    w2 = W(1).reshape(64, 64, 3, 3)
    pair = np.concatenate([w2[:, :, :, 0], w2[:, :, :, 1]], axis=0)  # [128, co, dy]
    ws[0:128, W2P_OFF : W2P_OFF + 192] = pair.transpose(0, 2, 1).reshape(128, 192)
    ws[0:64, W2S_OFF : W2S_OFF + 192] = (
        w2[:, :, :, 2].transpose(0, 2, 1).reshape(64, 192)
    )
    # L2 pair/single (cout=128)
    w3 = W(2).reshape(64, 128, 3, 3)
    pair = np.concatenate([w3[:, :, :, 0], w3[:, :, :, 1]], axis=0)
    ws[0:128, W3P_OFF : W3P_OFF + 384] = pair.transpose(0, 2, 1).reshape(128, 384)
    ws[0:64, W3S_OFF : W3S_OFF + 384] = (
        w3[:, :, :, 2].transpose(0, 2, 1).reshape(64, 384)
    )
    # L3: [128, 9*128]
    ws[0:128, W4_OFF : W4_OFF + 1152] = W(3).transpose(0, 2, 1).reshape(128, 1152)
    # FC3 weights [512, 10, 1] -> [128, 4, 10]
    wf3 = W(15)[:, :, 0].reshape(4, 128, 10).transpose(1, 0, 2)
    ws[0:128, WFC3_OFF : WFC3_OFF + 40] = wf3.reshape(128, 40)
    d["wsmall"] = ws.astype(np.float16)

    bb = np.zeros((128, BIAS_COLS), np.float32)
    for j in range(13):
        co = LAYERS[j][1]
        t = max(1, co // 128)
        bb[: min(co, 128), _BCOLS[j] : _BCOLS[j] + t] = B(j).reshape(t, -1).T
    bb[:, BFC1_COL : BFC1_COL + 4] = B(13).reshape(4, 128).T
    bb[:, BFC2_COL : BFC2_COL + 4] = B(14).reshape(4, 128).T
    d["wbias"] = bb
    d["bfc3"] = B(15).reshape(1, 10).astype(np.float16)

    for j in range(4, 13):
        w = W(j)  # [cin, cout, 9]
        cin, cout = w.shape[0], w.shape[1]
        T = cin // 128
        blob = w.reshape(T, 128, cout, 9).transpose(1, 0, 3, 2)  # [128, T, 9, cout]
        for t in range(T):
            d[f"w_l{j}_{t}"] = np.ascontiguousarray(
                blob[:, t].reshape(128, 9 * cout)
            ).astype(np.float16)

    for k, j in [("wfc1", 13), ("wfc2", 14)]:
        w = W(j)[:, :, 0]  # [512, 512]
        d[k] = np.ascontiguousarray(
            w.reshape(4, 128, 512).transpose(1, 0, 2).reshape(128, 4 * 512)
        ).astype(np.float16)
    return d


def kernel(x, weights, biases):
    nc = build_nc()
    in_maps = [prep_core_inputs(c, x, weights, biases) for c in range(N_CORES)]
    res = run_bass_kernel_spmd(nc, in_maps, list(range(N_CORES)))
    out = np.zeros((MODEL_BS, IMG_BS, 10), np.float32)
    for c in range(N_CORES):
        m, h = c // 2, c % 2
        out[m, h * N : (h + 1) * N] = res.results[c]["y"]
    return out
